# revision 9
# baseline (speedup 1.0000x reference)
"""SMPL body-model (shape/pose blendshapes + LBS) on 8 TRN2 NeuronCores.

Data-parallel over batch: 1024/8 = 128 rows per core, batch on SBUF
partitions everywhere. Heavy matmuls run in fp32r; the kinematic chain and
the per-vertex transform apply run on the vector engine in fp32.
"""

import sys

sys.path.insert(0, "/opt/trn_rl_repo")

import math
from contextlib import ExitStack

import numpy as np

import concourse.bass as bass
import concourse.tile as tile
from concourse import bacc, mybir
from concourse.bass_utils import run_bass_kernel_spmd

F32 = mybir.dt.float32
F32R = mybir.dt.float32r
ALU = mybir.AluOpType
AFT = mybir.ActivationFunctionType

B, V, J = 1024, 6890, 24
NCORES, BL = 8, 128
PF = (J - 1) * 9  # 207
VK = V * 3  # 20670
CH_A = 1024  # stage A chunk (columns of v*3, padded)
CH_B = 512  # stage B chunk (vertices, padded)
VP = 7168  # padded vertex count (14 * 512)
VKP = VP * 3  # 21504 = 21 * 1024
NA = VKP // CH_A  # 21 stage A chunks
NB = VP // CH_B  # 14 stage B chunks
SMPL_PARENTS = [-1, 0, 0, 0, 1, 2, 3, 4, 5, 6, 7, 8, 9, 9, 9, 12, 13, 14, 16, 17, 18, 19, 20, 21]

# (j0, nj, p0, pstep) groups with affine parent indexing, grouped by tree depth
CHAIN_GROUPS = [
    (1, 3, 0, 0),
    (4, 3, 1, 1),
    (7, 3, 4, 1),
    (10, 3, 7, 1),
    (13, 2, 9, 0),
    (15, 3, 12, 1),
    (18, 2, 16, 1),
    (20, 2, 18, 1),
    (22, 2, 20, 1),
]
# rel_joints only needs J_rest, so consecutive-parent runs can span depths
REL_GROUPS = [(1, 3, 0, 0), (4, 9, 1, 1), (13, 2, 9, 0), (15, 3, 12, 1), (18, 6, 16, 1)]


def _jsl(view, j0, nj, step):
    """view[:, j0 : j0+nj] with parent step 1 or broadcast (step 0)."""
    if step == 1:
        return view[:, j0 : j0 + nj]
    sl = view[:, j0 : j0 + 1]
    return sl.broadcast_to((sl.shape[0], nj) + tuple(sl.shape[2:]))


def build_nc(debug=False):
    nc = bacc.Bacc("TRN2", target_bir_lowering=False, debug=False, num_devices=NCORES)

    pose_d = nc.dram_tensor("pose", [BL, 72], F32, kind="ExternalInput")
    ba_d = nc.dram_tensor("betas_aug", [BL, 12], F32, kind="ExternalInput")
    js2_d = nc.dram_tensor("js2aug", [12, 72], F32, kind="ExternalInput")
    sdir_d = nc.dram_tensor("sdirT_aug", [NA, 12, CH_A], F32, kind="ExternalInput")
    pdir_d = nc.dram_tensor("posedirs", [NA, PF, CH_A], F32, kind="ExternalInput")
    wt_d = nc.dram_tensor("wT", [128, VP], F32, kind="ExternalInput")
    id_d = nc.dram_tensor("ident", [128, 128], F32, kind="ExternalInput")

    verts_d = nc.dram_tensor("verts", [NB, BL, 3 * CH_B], F32, kind="ExternalOutput")
    vshaped_d = nc.dram_tensor("v_shaped", [NA, BL, CH_A], F32, kind="ExternalOutput")
    jposed_d = nc.dram_tensor("j_posed", [BL, 72], F32, kind="ExternalOutput")
    jrest_d = nc.dram_tensor("j_rest", [BL, 72], F32, kind="ExternalOutput")
    a_d = nc.dram_tensor("a_mats", [BL, 384], F32, kind="ExternalOutput")
    if debug:
        rot_dbg = nc.dram_tensor("rot_dbg", [BL, 216], F32, kind="ExternalOutput")
        rg_dbg = nc.dram_tensor("rg_dbg", [BL, 216], F32, kind="ExternalOutput")

    with tile.TileContext(nc) as tc, ExitStack() as ctx:
        keep = ctx.enter_context(tc.tile_pool(name="keep", bufs=1))
        sm = ctx.enter_context(tc.tile_pool(name="small", bufs=1))

        # ---------------- loads ----------------
        pose_t = sm.tile([BL, 72], F32)
        nc.sync.dma_start(pose_t[:], pose_d[:])
        ba_t = sm.tile([BL, 12], F32)
        nc.sync.dma_start(ba_t[:], ba_d[:])
        id_t = keep.tile([128, 128], F32)
        nc.sync.dma_start(id_t[:], id_d[:])
        js2_t = sm.tile([12, 72], F32)
        nc.sync.dma_start(js2_t[:], js2_d[:])
        wt_t = keep.tile([128, VP], F32R)
        nc.sync.dma_start(wt_t[:], wt_d[:].bitcast(F32R))

        vp_buf = keep.tile([BL, VKP], F32)  # v_posed, resident (padded)
        btT_r = keep.tile([12, BL], F32R)
        poseT1 = keep.tile([128, BL], F32R)
        poseT2 = keep.tile([PF - 128, BL], F32R)
        amnT = keep.tile([128, 3 * BL], F32R)  # per m: 4 row-packed [24,128] lhsT

        with tc.tile_pool(name="ps0", bufs=1, space="PSUM") as ps0:
            # betas^T (for J_rest fp32 matmul and stage A fp32r lhsT)
            ps_bt = ps0.tile([12, BL], F32)
            nc.tensor.transpose(ps_bt[:], ba_t[:], id_t[:])
            btT_f = sm.tile([12, BL], F32)
            nc.scalar.copy(btT_f[:], ps_bt[:])
            nc.vector.tensor_copy(btT_r[:], ps_bt[:])

            # J_rest = betas_aug @ js2aug  (exact fp32)
            ps_jr = ps0.tile([BL, 72], F32)
            nc.tensor.matmul(ps_jr[:], btT_f[:], js2_t[:], start=True, stop=True)
            jrest_t = sm.tile([BL, 72], F32)
            nc.scalar.copy(jrest_t[:], ps_jr[:])
            nc.sync.dma_start(jrest_d[:], jrest_t[:])

            # ---------------- Rodrigues ----------------
            rv8 = sm.tile([BL, 72], F32)
            nc.vector.tensor_scalar_add(rv8[:], pose_t[:], 1e-8)
            sq = sm.tile([BL, 72], F32)
            nc.vector.tensor_mul(sq[:], rv8[:], rv8[:])
            n2 = sm.tile([BL, J], F32)
            nc.vector.reduce_sum(
                n2[:], sq[:].rearrange("p (j k) -> p j k", k=3), axis=mybir.AxisListType.X
            )
            ang = sm.tile([BL, J], F32)
            nc.scalar.activation(ang[:], n2[:], AFT.Sqrt)
            inv = sm.tile([BL, J], F32)
            nc.vector.reciprocal(inv[:], ang[:])
            s_t = sm.tile([BL, J], F32)
            nc.scalar.activation(s_t[:], ang[:], AFT.Sin)
            c_t = sm.tile([BL, J], F32)
            halfpi = sm.tile([BL, 1], F32)
            nc.vector.memset(halfpi[:], math.pi / 2)
            nc.scalar.activation(c_t[:], ang[:], AFT.Sin, bias=halfpi[:])

            axis_t = sm.tile([BL, 72], F32)  # [b, (j,3)]
            ax3 = axis_t[:].rearrange("p (j k) -> p j k", k=3)
            nc.vector.tensor_mul(
                ax3,
                pose_t[:].rearrange("p (j k) -> p j k", k=3),
                inv[:].unsqueeze(2).broadcast_to([BL, J, 3]),
            )

            rot = sm.tile([BL, 216], F32)  # [b, (j,m,n)] local rotations
            r4 = rot[:].rearrange("p (j m n) -> p j m n", m=3, n=3)
            # (1-c) * outer(axis, axis)
            omc = sm.tile([BL, J], F32)
            nc.vector.tensor_scalar(omc[:], c_t[:], -1.0, 1.0, op0=ALU.mult, op1=ALU.add)
            nc.vector.tensor_mul(
                r4,
                ax3.unsqueeze(3).broadcast_to([BL, J, 3, 3]),
                ax3.unsqueeze(2).broadcast_to([BL, J, 3, 3]),
            )
            r9 = rot[:].rearrange("p (j a) -> p j a", a=9)
            nc.vector.tensor_mul(
                r9, r9, omc[:].unsqueeze(2).broadcast_to([BL, J, 9])
            )
            # + c on the diagonal
            nc.vector.tensor_add(
                r9[:, :, 0:9:4], r9[:, :, 0:9:4], c_t[:].unsqueeze(2).broadcast_to([BL, J, 3])
            )
            # +/- s*axis off-diagonals
            sa = sm.tile([BL, 72], F32)
            sa3 = sa[:].rearrange("p (j k) -> p j k", k=3)
            nc.vector.tensor_mul(sa3, ax3, s_t[:].unsqueeze(2).broadcast_to([BL, J, 3]))
            for pos, comp, sign in ((1, 2, -1), (3, 2, 1), (2, 1, 1), (6, 1, -1), (5, 0, -1), (7, 0, 1)):
                op = nc.vector.tensor_add if sign > 0 else nc.vector.tensor_sub
                op(r9[:, :, pos], r9[:, :, pos], sa3[:, :, comp])
            if debug:
                rot_sb = sm.tile([BL, 216], F32)
                nc.vector.tensor_copy(rot_sb[:], rot[:])
                nc.sync.dma_start(rot_dbg[:], rot_sb[:])

            # ---------------- rel joints ----------------
            rel = sm.tile([BL, 72], F32)
            jr3 = jrest_t[:].rearrange("p (j k) -> p j k", k=3)
            rl3 = rel[:].rearrange("p (j k) -> p j k", k=3)
            nc.vector.tensor_copy(rl3[:, 0:1], jr3[:, 0:1])
            for j0, nj, p0, pstep in REL_GROUPS:
                nc.vector.tensor_sub(
                    rl3[:, j0 : j0 + nj], jr3[:, j0 : j0 + nj], _jsl(jr3, p0, nj, pstep)
                )

            # ---------------- kinematic chain ----------------
            rg = sm.tile([BL, 216], F32)
            tg = sm.tile([BL, 72], F32)
            g4 = rg[:].rearrange("p (j m n) -> p j m n", m=3, n=3)
            t3 = tg[:].rearrange("p (j k) -> p j k", k=3)
            nc.vector.tensor_copy(rg[:, 0:9], rot[:, 0:9])
            nc.vector.tensor_copy(t3[:, 0:1], rl3[:, 0:1])
            tmpR = sm.tile([BL, 27], F32)
            tmpt = sm.tile([BL, 9], F32)
            for j0, nj, p0, pstep in CHAIN_GROUPS:
                dstR = g4[:, j0 : j0 + nj]  # [b, nj, m, n]
                locR = r4[:, j0 : j0 + nj]
                parR = _jsl(g4, p0, nj, pstep)
                tR = tmpR[:].rearrange("p (j m n) -> p j m n", m=3, n=3)[:, 0:nj]
                for k in range(3):
                    a_in = parR[:, :, :, k].unsqueeze(3).broadcast_to([BL, nj, 3, 3])
                    b_in = locR[:, :, k, :].unsqueeze(2).broadcast_to([BL, nj, 3, 3])
                    if k == 0:
                        nc.vector.tensor_mul(dstR, a_in, b_in)
                    else:
                        nc.vector.tensor_mul(tR, a_in, b_in)
                        nc.vector.tensor_add(dstR, dstR, tR)
                # translations
                dstT = t3[:, j0 : j0 + nj]
                locT = rl3[:, j0 : j0 + nj]
                tT = tmpt[:].rearrange("p (j k) -> p j k", k=3)[:, 0:nj]
                for k in range(3):
                    a_in = parR[:, :, :, k]  # [b, nj, 3(m)]
                    b_in = locT[:, :, k].unsqueeze(2).broadcast_to([BL, nj, 3])
                    if k == 0:
                        nc.vector.tensor_mul(dstT, a_in, b_in)
                    else:
                        nc.vector.tensor_mul(tT, a_in, b_in)
                        nc.vector.tensor_add(dstT, dstT, tT)
                nc.vector.tensor_add(dstT, dstT, _jsl(t3, p0, nj, pstep))

            nc.sync.dma_start(jposed_d[:], tg[:])
            if debug:
                rg_sb = sm.tile([BL, 216], F32)
                nc.vector.tensor_copy(rg_sb[:], rg[:])
                nc.sync.dma_start(rg_dbg[:], rg_sb[:])

            # ---------------- A matrices ----------------
            # ta = tg - sum_k Rg[:,:, :,k] * J_rest[:,:,k]
            ta = sm.tile([BL, 72], F32)
            ta3 = ta[:].rearrange("p (j k) -> p j k", k=3)
            acc = sm.tile([BL, 72], F32)
            acc3 = acc[:].rearrange("p (j k) -> p j k", k=3)
            for k in range(3):
                a_in = g4[:, :, :, k]  # [b, J, 3(m)]
                b_in = jr3[:, :, k].unsqueeze(2).broadcast_to([BL, J, 3])
                if k == 0:
                    nc.vector.tensor_mul(acc3, a_in, b_in)
                else:
                    nc.vector.tensor_mul(ta3, a_in, b_in)
                    nc.vector.tensor_add(acc3, acc3, ta3)
            nc.vector.tensor_sub(ta3, t3, acc3)

            a_full = sm.tile([BL, 384], F32)
            a4 = a_full[:].rearrange("p (j m n) -> p j m n", m=4, n=4)
            nc.vector.memset(a_full[:], 0.0)
            nc.vector.memset(a4[:, :, 3, 3], 1.0)
            nc.vector.tensor_copy(a4[:, :, 0:3, 0:3], g4)
            nc.vector.tensor_copy(a4[:, :, 0:3, 3], ta3)
            nc.sync.dma_start(a_d[:], a_full[:])

            # ---------------- pose_feature^T ----------------
            nc.vector.tensor_scalar_add(
                rot[:, 9:216].rearrange("p (j a) -> p j a", a=9)[:, :, 0:9:4],
                rot[:, 9:216].rearrange("p (j a) -> p j a", a=9)[:, :, 0:9:4],
                -1.0,
            )
            ps_p1 = ps0.tile([128, BL], F32)
            nc.tensor.transpose(ps_p1[:], rot[:, 9:137], id_t[:])
            nc.vector.tensor_copy(poseT1[:], ps_p1[:])
            ps_p2 = ps0.tile([PF - 128, BL], F32)
            nc.tensor.transpose(ps_p2[:], rot[:, 137:216], id_t[:])
            nc.vector.tensor_copy(poseT2[:], ps_p2[:])

        # ---------------- A^T slices for LBS ----------------
        with (
            tc.tile_pool(name="psT", bufs=3, space="PSUM") as psT,
            tc.tile_pool(name="trT", bufs=3) as trT,
        ):
            for m in range(3):
                for n in range(4):
                    ps_t = psT.tile([J, BL], F32, tag="ps_t")
                    nc.tensor.transpose(ps_t[:], a4[:, :, m, n], id_t[:])
                    tr_t = trT.tile([J, BL], F32R, tag="tr_t")
                    nc.scalar.copy(tr_t[:], ps_t[:])
                    nc.sync.dma_start(
                        amnT[32 * n : 32 * n + J, m * BL : (m + 1) * BL], tr_t[:]
                    )

        # ---------------- stage A: v_shaped + v_posed ----------------
        with (
            tc.tile_pool(name="streamA", bufs=3) as stA,
            tc.tile_pool(name="outA", bufs=3) as outA,
            tc.tile_pool(name="psA", bufs=2, space="PSUM") as psA,
        ):
            for ci in range(NA):
                c0 = ci * CH_A
                sd = stA.tile([12, CH_A], F32R, tag="sd")
                nc.sync.dma_start(sd[:], sdir_d[ci].bitcast(F32R))
                pd1 = stA.tile([128, CH_A], F32R, tag="pd1")
                nc.sync.dma_start(pd1[:], pdir_d[ci, 0:128].bitcast(F32R))
                pd2 = stA.tile([PF - 128, CH_A], F32R, tag="pd2")
                nc.sync.dma_start(pd2[:], pdir_d[ci, 128:PF].bitcast(F32R))

                ps_vs = psA.tile([BL, CH_A], F32, tag="vs")
                ps_vp = psA.tile([BL, CH_A], F32, tag="vp")
                for h0 in range(0, CH_A, 512):
                    hs = slice(h0, h0 + 512)
                    nc.tensor.matmul(ps_vs[:, hs], btT_r[:], sd[:, hs], start=True, stop=True)
                    nc.tensor.matmul(ps_vp[:, hs], poseT1[:], pd1[:, hs], start=True, stop=False)
                    nc.tensor.matmul(ps_vp[:, hs], poseT2[:], pd2[:, hs], start=False, stop=True)
                vs_sb = outA.tile([BL, CH_A], F32, tag="vs_sb")
                nc.scalar.copy(vs_sb[:], ps_vs[:])
                nc.sync.dma_start(vshaped_d[ci], vs_sb[:])
                nc.vector.tensor_add(vp_buf[:, c0 : c0 + CH_A], ps_vp[:], vs_sb[:])

        # ---------------- stage B: LBS ----------------
        vp3 = vp_buf[:].rearrange("p (v k) -> p v k", k=3)
        with (
            tc.tile_pool(name="outB", bufs=3) as outB,
            tc.tile_pool(name="tmpB", bufs=4) as tmpB,
            tc.tile_pool(name="psB", bufs=2, space="PSUM") as psB,
        ):
            for ci in range(NB):
                c0 = ci * CH_B
                verts_sb = outB.tile([BL, 3 * CH_B], F32, tag="verts")
                vv = verts_sb[:].rearrange("p (v m) -> p v m", m=3)
                for m in range(3):
                    ps_T = psB.tile([BL, 4, CH_B], F32, tag="T")
                    for n in range(4):
                        nc.tensor.matmul(
                            ps_T[:, n, :],
                            amnT[32 * n : 32 * n + J, m * BL : (m + 1) * BL],
                            wt_t[32 * n : 32 * n + J, c0 : c0 + CH_B],
                            start=True,
                            stop=True,
                            tile_position=(32 * n, 0),
                        )
                    p0 = tmpB.tile([BL, CH_B], F32, tag="p0")
                    nc.vector.tensor_mul(p0[:], ps_T[:, 0, :], vp3[:, c0 : c0 + CH_B, 0])
                    p1 = tmpB.tile([BL, CH_B], F32, tag="p1")
                    nc.vector.tensor_mul(p1[:], ps_T[:, 1, :], vp3[:, c0 : c0 + CH_B, 1])
                    p2 = tmpB.tile([BL, CH_B], F32, tag="p2")
                    nc.vector.tensor_mul(p2[:], ps_T[:, 2, :], vp3[:, c0 : c0 + CH_B, 2])
                    s0 = tmpB.tile([BL, CH_B], F32, tag="s0")
                    nc.gpsimd.tensor_add(s0[:], p0[:], p1[:])
                    nc.gpsimd.tensor_add(s0[:], s0[:], p2[:])
                    nc.vector.tensor_add(vv[:, :, m], ps_T[:, 3, :], s0[:])
                nc.sync.dma_start(verts_d[ci], verts_sb[:])

    nc.finalize()
    return nc


_NC_CACHE = {}


def _get_nc(debug=False):
    key = bool(debug)
    if key not in _NC_CACHE:
        _NC_CACHE[key] = build_nc(debug=debug)
    return _NC_CACHE[key]


def _host_prep(betas, full_pose, v_template, shapedirs, posedirs, J_regressor, lbs_weights):
    betas = np.asarray(betas, np.float32)
    full_pose = np.asarray(full_pose, np.float32)
    v_template = np.asarray(v_template, np.float32)
    shapedirs = np.asarray(shapedirs, np.float32)
    posedirs = np.asarray(posedirs, np.float32)
    J_regressor = np.asarray(J_regressor, np.float32)
    lbs_weights = np.asarray(lbs_weights, np.float32)

    sd_flat = shapedirs.reshape(VK, 10)
    vt_flat = v_template.reshape(VK)
    vt_hi = vt_flat.astype(np.dtype("bfloat16") if hasattr(np, "bfloat16") else np.float32)
    import ml_dtypes

    vt_hi = vt_flat.astype(ml_dtypes.bfloat16).astype(np.float32)
    vt_lo = vt_flat - vt_hi
    sdirT_aug = np.zeros((12, VKP), np.float32)
    sdirT_aug[0:10, 0:VK] = sd_flat.T
    sdirT_aug[10, 0:VK] = vt_hi
    sdirT_aug[11, 0:VK] = vt_lo
    sdirT_aug = np.ascontiguousarray(
        sdirT_aug.reshape(12, NA, CH_A).transpose(1, 0, 2)
    )  # [NA, 12, CH_A]
    jrs = np.einsum(
        "jv,vkl->ljk", J_regressor.astype(np.float64), shapedirs.astype(np.float64)
    ).reshape(10, 72)
    jt = (J_regressor.astype(np.float64) @ v_template[0].astype(np.float64)).reshape(1, 72)
    js2aug = np.ascontiguousarray(
        np.concatenate([jrs, jt, np.zeros((1, 72))], axis=0), np.float32
    )  # [12, 72]
    wT = np.zeros((128, VP), np.float32)
    for n in range(4):
        wT[32 * n : 32 * n + J, 0:V] = lbs_weights.T
    betas_aug = np.concatenate([betas, np.ones((B, 2), np.float32)], axis=1)  # [B, 12]
    ident = np.eye(128, dtype=np.float32)
    return betas_aug, full_pose, sdirT_aug, js2aug, wT, ident


def kernel(betas, full_pose, v_template, shapedirs, posedirs, J_regressor, lbs_weights, parents):
    betas_aug, full_pose, sdirT_aug, js2aug, wT, ident = _host_prep(
        betas, full_pose, v_template, shapedirs, posedirs, J_regressor, lbs_weights
    )
    pd = np.asarray(posedirs, np.float32)
    pdt = np.zeros((PF, VKP), np.float32)
    pdt[:, 0:VK] = pd
    posedirs = np.ascontiguousarray(pdt.reshape(PF, NA, CH_A).transpose(1, 0, 2))  # [NA, PF, CH_A]

    nc = _get_nc(debug=False)
    in_maps = []
    for i in range(NCORES):
        sl = slice(i * BL, (i + 1) * BL)
        in_maps.append(
            {
                "pose": np.ascontiguousarray(full_pose[sl]),
                "betas_aug": np.ascontiguousarray(betas_aug[sl]),
                "js2aug": js2aug,
                "sdirT_aug": sdirT_aug,
                "posedirs": posedirs,
                "wT": wT,
                "ident": ident,
            }
        )
    res = run_bass_kernel_spmd(nc, in_maps, list(range(NCORES)))

    verts = np.empty((B, V, 3), np.float32)
    v_shaped = np.empty((B, V, 3), np.float32)
    j_posed = np.empty((B, J, 3), np.float32)
    j_rest = np.empty((B, J, 3), np.float32)
    a_mats = np.empty((B, J, 4, 4), np.float32)
    for i, r in enumerate(res.results):
        sl = slice(i * BL, (i + 1) * BL)
        verts[sl] = (
            r["verts"].transpose(1, 0, 2).reshape(BL, VP, 3)[:, 0:V, :]
        )
        v_shaped[sl] = (
            r["v_shaped"].transpose(1, 0, 2).reshape(BL, VKP)[:, 0:VK].reshape(BL, V, 3)
        )
        j_posed[sl] = r["j_posed"].reshape(BL, J, 3)
        j_rest[sl] = r["j_rest"].reshape(BL, J, 3)
        a_mats[sl] = r["a_mats"].reshape(BL, J, 4, 4)
    return verts, j_posed, j_rest, a_mats, v_shaped


# revision 11
# speedup vs baseline: 1.2123x; 1.2123x over previous
"""SMPL body-model (shape/pose blendshapes + LBS) on 8 TRN2 NeuronCores.

Data-parallel over batch: 1024/8 = 128 rows per core, batch on SBUF
partitions everywhere. Heavy matmuls run in fp32r; the kinematic chain and
the per-vertex transform apply run on the vector engine in fp32.
"""

import sys

sys.path.insert(0, "/opt/trn_rl_repo")

import math
from contextlib import ExitStack

import numpy as np

import concourse.bass as bass
import concourse.tile as tile
from concourse import bacc, mybir
from concourse.bass_utils import run_bass_kernel_spmd

F32 = mybir.dt.float32
F32R = mybir.dt.float32r
ALU = mybir.AluOpType
AFT = mybir.ActivationFunctionType

B, V, J = 1024, 6890, 24
NCORES, BL = 8, 128
PF = (J - 1) * 9  # 207
VK = V * 3  # 20670
CH_A = 1024  # stage A chunk (columns of v*3, padded)
CH_B = 512  # stage B chunk (vertices, padded)
VP = 7168  # padded vertex count (14 * 512)
VKP = VP * 3  # 21504 = 21 * 1024
NA = VKP // CH_A  # 21 stage A chunks
NB = VP // CH_B  # 14 stage B chunks
SMPL_PARENTS = [-1, 0, 0, 0, 1, 2, 3, 4, 5, 6, 7, 8, 9, 9, 9, 12, 13, 14, 16, 17, 18, 19, 20, 21]

# (j0, nj, p0, pstep) groups with affine parent indexing, grouped by tree depth
CHAIN_GROUPS = [
    (1, 3, 0, 0),
    (4, 3, 1, 1),
    (7, 3, 4, 1),
    (10, 3, 7, 1),
    (13, 2, 9, 0),
    (15, 3, 12, 1),
    (18, 2, 16, 1),
    (20, 2, 18, 1),
    (22, 2, 20, 1),
]
# rel_joints only needs J_rest, so consecutive-parent runs can span depths
REL_GROUPS = [(1, 3, 0, 0), (4, 9, 1, 1), (13, 2, 9, 0), (15, 3, 12, 1), (18, 6, 16, 1)]


def _jsl(view, j0, nj, step):
    """view[:, j0 : j0+nj] with parent step 1 or broadcast (step 0)."""
    if step == 1:
        return view[:, j0 : j0 + nj]
    sl = view[:, j0 : j0 + 1]
    return sl.broadcast_to((sl.shape[0], nj) + tuple(sl.shape[2:]))


def build_nc(debug=False):
    nc = bacc.Bacc("TRN2", target_bir_lowering=False, debug=False, num_devices=NCORES)

    pose_d = nc.dram_tensor("pose", [BL, 72], F32, kind="ExternalInput")
    ba_d = nc.dram_tensor("betas_aug", [BL, 12], F32, kind="ExternalInput")
    js2_d = nc.dram_tensor("js2aug", [12, 72], F32, kind="ExternalInput")
    sdir_d = nc.dram_tensor("sdirT_aug", [NA, 12, CH_A], F32, kind="ExternalInput")
    pdir_d = nc.dram_tensor("posedirs", [NA, PF, CH_A], F32, kind="ExternalInput")
    wt_d = nc.dram_tensor("wT", [128, VP], F32, kind="ExternalInput")
    id_d = nc.dram_tensor("ident", [128, 128], F32, kind="ExternalInput")

    verts_d = nc.dram_tensor("verts", [NB, BL, 3 * CH_B], F32, kind="ExternalOutput")
    vshaped_d = nc.dram_tensor("v_shaped", [NA, BL, CH_A], F32, kind="ExternalOutput")
    jposed_d = nc.dram_tensor("j_posed", [BL, 72], F32, kind="ExternalOutput")
    jrest_d = nc.dram_tensor("j_rest", [BL, 72], F32, kind="ExternalOutput")
    a_d = nc.dram_tensor("a_mats", [BL, 384], F32, kind="ExternalOutput")
    if debug:
        rot_dbg = nc.dram_tensor("rot_dbg", [BL, 216], F32, kind="ExternalOutput")
        rg_dbg = nc.dram_tensor("rg_dbg", [BL, 216], F32, kind="ExternalOutput")

    with tile.TileContext(nc) as tc, ExitStack() as ctx:
        keep = ctx.enter_context(tc.tile_pool(name="keep", bufs=1))
        sm = ctx.enter_context(tc.tile_pool(name="small", bufs=1))

        # ---------------- loads ----------------
        pose_t = sm.tile([BL, 72], F32)
        nc.sync.dma_start(pose_t[:], pose_d[:])
        ba_t = sm.tile([BL, 12], F32)
        nc.sync.dma_start(ba_t[:], ba_d[:])
        id_t = keep.tile([128, 128], F32)
        nc.sync.dma_start(id_t[:], id_d[:])
        js2_t = sm.tile([12, 72], F32)
        nc.sync.dma_start(js2_t[:], js2_d[:])
        wt_t = keep.tile([128, VP], F32R)
        nc.sync.dma_start(wt_t[:], wt_d[:].bitcast(F32R))

        vp_buf = keep.tile([BL, VKP], F32)  # v_posed, resident (padded)
        btT_r = keep.tile([12, BL], F32R)
        poseT1 = keep.tile([128, BL], F32R)
        poseT2 = keep.tile([PF - 128, BL], F32R)
        amnT = keep.tile([128, 3 * BL], F32R)  # per m: 4 row-packed [24,128] lhsT

        with tc.tile_pool(name="ps0", bufs=1, space="PSUM") as ps0:
            # betas^T (for J_rest fp32 matmul and stage A fp32r lhsT)
            ps_bt = ps0.tile([12, BL], F32)
            nc.tensor.transpose(ps_bt[:], ba_t[:], id_t[:])
            btT_f = sm.tile([12, BL], F32)
            nc.scalar.copy(btT_f[:], ps_bt[:])
            nc.vector.tensor_copy(btT_r[:], ps_bt[:])

            # J_rest = betas_aug @ js2aug  (exact fp32)
            ps_jr = ps0.tile([BL, 72], F32)
            nc.tensor.matmul(ps_jr[:], btT_f[:], js2_t[:], start=True, stop=True)
            jrest_t = sm.tile([BL, 72], F32)
            nc.scalar.copy(jrest_t[:], ps_jr[:])
            nc.sync.dma_start(jrest_d[:], jrest_t[:])

            # ---------------- Rodrigues ----------------
            rv8 = sm.tile([BL, 72], F32)
            nc.vector.tensor_scalar_add(rv8[:], pose_t[:], 1e-8)
            sq = sm.tile([BL, 72], F32)
            nc.vector.tensor_mul(sq[:], rv8[:], rv8[:])
            n2 = sm.tile([BL, J], F32)
            nc.vector.reduce_sum(
                n2[:], sq[:].rearrange("p (j k) -> p j k", k=3), axis=mybir.AxisListType.X
            )
            ang = sm.tile([BL, J], F32)
            nc.scalar.activation(ang[:], n2[:], AFT.Sqrt)
            inv = sm.tile([BL, J], F32)
            nc.vector.reciprocal(inv[:], ang[:])
            s_t = sm.tile([BL, J], F32)
            nc.scalar.activation(s_t[:], ang[:], AFT.Sin)
            c_t = sm.tile([BL, J], F32)
            halfpi = sm.tile([BL, 1], F32)
            nc.vector.memset(halfpi[:], math.pi / 2)
            nc.scalar.activation(c_t[:], ang[:], AFT.Sin, bias=halfpi[:])

            axis_t = sm.tile([BL, 72], F32)  # [b, (j,3)]
            ax3 = axis_t[:].rearrange("p (j k) -> p j k", k=3)
            nc.vector.tensor_mul(
                ax3,
                pose_t[:].rearrange("p (j k) -> p j k", k=3),
                inv[:].unsqueeze(2).broadcast_to([BL, J, 3]),
            )

            rot = sm.tile([BL, 216], F32)  # [b, (j,m,n)] local rotations
            r4 = rot[:].rearrange("p (j m n) -> p j m n", m=3, n=3)
            # (1-c) * outer(axis, axis)
            omc = sm.tile([BL, J], F32)
            nc.vector.tensor_scalar(omc[:], c_t[:], -1.0, 1.0, op0=ALU.mult, op1=ALU.add)
            nc.vector.tensor_mul(
                r4,
                ax3.unsqueeze(3).broadcast_to([BL, J, 3, 3]),
                ax3.unsqueeze(2).broadcast_to([BL, J, 3, 3]),
            )
            r9 = rot[:].rearrange("p (j a) -> p j a", a=9)
            nc.vector.tensor_mul(
                r9, r9, omc[:].unsqueeze(2).broadcast_to([BL, J, 9])
            )
            # + c on the diagonal
            nc.vector.tensor_add(
                r9[:, :, 0:9:4], r9[:, :, 0:9:4], c_t[:].unsqueeze(2).broadcast_to([BL, J, 3])
            )
            # +/- s*axis off-diagonals
            sa = sm.tile([BL, 72], F32)
            sa3 = sa[:].rearrange("p (j k) -> p j k", k=3)
            nc.vector.tensor_mul(sa3, ax3, s_t[:].unsqueeze(2).broadcast_to([BL, J, 3]))
            for pos, comp, sign in ((1, 2, -1), (3, 2, 1), (2, 1, 1), (6, 1, -1), (5, 0, -1), (7, 0, 1)):
                op = nc.vector.tensor_add if sign > 0 else nc.vector.tensor_sub
                op(r9[:, :, pos], r9[:, :, pos], sa3[:, :, comp])
            if debug:
                rot_sb = sm.tile([BL, 216], F32)
                nc.vector.tensor_copy(rot_sb[:], rot[:])
                nc.sync.dma_start(rot_dbg[:], rot_sb[:])

            # ---------------- rel joints ----------------
            rel = sm.tile([BL, 72], F32)
            jr3 = jrest_t[:].rearrange("p (j k) -> p j k", k=3)
            rl3 = rel[:].rearrange("p (j k) -> p j k", k=3)
            nc.vector.tensor_copy(rl3[:, 0:1], jr3[:, 0:1])
            for j0, nj, p0, pstep in REL_GROUPS:
                nc.vector.tensor_sub(
                    rl3[:, j0 : j0 + nj], jr3[:, j0 : j0 + nj], _jsl(jr3, p0, nj, pstep)
                )

            # ---------------- kinematic chain ----------------
            rg = sm.tile([BL, 216], F32)
            tg = sm.tile([BL, 72], F32)
            g4 = rg[:].rearrange("p (j m n) -> p j m n", m=3, n=3)
            t3 = tg[:].rearrange("p (j k) -> p j k", k=3)
            nc.vector.tensor_copy(rg[:, 0:9], rot[:, 0:9])
            nc.vector.tensor_copy(t3[:, 0:1], rl3[:, 0:1])
            tmpR = sm.tile([BL, 27], F32)
            tmpt = sm.tile([BL, 9], F32)
            for j0, nj, p0, pstep in CHAIN_GROUPS:
                dstR = g4[:, j0 : j0 + nj]  # [b, nj, m, n]
                locR = r4[:, j0 : j0 + nj]
                parR = _jsl(g4, p0, nj, pstep)
                tR = tmpR[:].rearrange("p (j m n) -> p j m n", m=3, n=3)[:, 0:nj]
                for k in range(3):
                    a_in = parR[:, :, :, k].unsqueeze(3).broadcast_to([BL, nj, 3, 3])
                    b_in = locR[:, :, k, :].unsqueeze(2).broadcast_to([BL, nj, 3, 3])
                    if k == 0:
                        nc.vector.tensor_mul(dstR, a_in, b_in)
                    else:
                        nc.vector.tensor_mul(tR, a_in, b_in)
                        nc.vector.tensor_add(dstR, dstR, tR)
                # translations
                dstT = t3[:, j0 : j0 + nj]
                locT = rl3[:, j0 : j0 + nj]
                tT = tmpt[:].rearrange("p (j k) -> p j k", k=3)[:, 0:nj]
                for k in range(3):
                    a_in = parR[:, :, :, k]  # [b, nj, 3(m)]
                    b_in = locT[:, :, k].unsqueeze(2).broadcast_to([BL, nj, 3])
                    if k == 0:
                        nc.vector.tensor_mul(dstT, a_in, b_in)
                    else:
                        nc.vector.tensor_mul(tT, a_in, b_in)
                        nc.vector.tensor_add(dstT, dstT, tT)
                nc.vector.tensor_add(dstT, dstT, _jsl(t3, p0, nj, pstep))

            nc.sync.dma_start(jposed_d[:], tg[:])
            if debug:
                rg_sb = sm.tile([BL, 216], F32)
                nc.vector.tensor_copy(rg_sb[:], rg[:])
                nc.sync.dma_start(rg_dbg[:], rg_sb[:])

            # ---------------- A matrices ----------------
            # ta = tg - sum_k Rg[:,:, :,k] * J_rest[:,:,k]
            ta = sm.tile([BL, 72], F32)
            ta3 = ta[:].rearrange("p (j k) -> p j k", k=3)
            acc = sm.tile([BL, 72], F32)
            acc3 = acc[:].rearrange("p (j k) -> p j k", k=3)
            for k in range(3):
                a_in = g4[:, :, :, k]  # [b, J, 3(m)]
                b_in = jr3[:, :, k].unsqueeze(2).broadcast_to([BL, J, 3])
                if k == 0:
                    nc.vector.tensor_mul(acc3, a_in, b_in)
                else:
                    nc.vector.tensor_mul(ta3, a_in, b_in)
                    nc.vector.tensor_add(acc3, acc3, ta3)
            nc.vector.tensor_sub(ta3, t3, acc3)

            a_full = sm.tile([BL, 384], F32)
            a4 = a_full[:].rearrange("p (j m n) -> p j m n", m=4, n=4)
            nc.vector.memset(a_full[:], 0.0)
            nc.vector.memset(a4[:, :, 3, 3], 1.0)
            nc.vector.tensor_copy(a4[:, :, 0:3, 0:3], g4)
            nc.vector.tensor_copy(a4[:, :, 0:3, 3], ta3)
            nc.sync.dma_start(a_d[:], a_full[:])

            # ---------------- pose_feature^T ----------------
            nc.vector.tensor_scalar_add(
                rot[:, 9:216].rearrange("p (j a) -> p j a", a=9)[:, :, 0:9:4],
                rot[:, 9:216].rearrange("p (j a) -> p j a", a=9)[:, :, 0:9:4],
                -1.0,
            )
            ps_p1 = ps0.tile([128, BL], F32)
            nc.tensor.transpose(ps_p1[:], rot[:, 9:137], id_t[:])
            nc.vector.tensor_copy(poseT1[:], ps_p1[:])
            ps_p2 = ps0.tile([PF - 128, BL], F32)
            nc.tensor.transpose(ps_p2[:], rot[:, 137:216], id_t[:])
            nc.vector.tensor_copy(poseT2[:], ps_p2[:])

        # ---------------- A^T slices for LBS ----------------
        with (
            tc.tile_pool(name="psT", bufs=3, space="PSUM") as psT,
            tc.tile_pool(name="trT", bufs=3) as trT,
        ):
            for m in range(3):
                for n in range(4):
                    ps_t = psT.tile([J, BL], F32, tag="ps_t")
                    nc.tensor.transpose(ps_t[:], a4[:, :, m, n], id_t[:])
                    tr_t = trT.tile([J, BL], F32R, tag="tr_t")
                    nc.scalar.copy(tr_t[:], ps_t[:])
                    nc.sync.dma_start(
                        amnT[32 * n : 32 * n + J, m * BL : (m + 1) * BL], tr_t[:]
                    )

        # ---------------- stages A (blendshapes) + B (LBS), interleaved ----------------
        vp3 = vp_buf[:].rearrange("p (v k) -> p v k", k=3)
        with (
            tc.tile_pool(name="streamA", bufs=3) as stA,
            tc.tile_pool(name="outA", bufs=3) as outA,
            tc.tile_pool(name="psA", bufs=2, space="PSUM") as psA,
            tc.tile_pool(name="outB", bufs=3) as outB,
            tc.tile_pool(name="tmpB", bufs=2) as tmpB,
            tc.tile_pool(name="psB", bufs=1, space="PSUM") as psB,
        ):

            def do_a(ci):
                c0 = ci * CH_A
                sd = stA.tile([12, CH_A], F32R, tag="sd")
                nc.sync.dma_start(sd[:], sdir_d[ci].bitcast(F32R))
                pd1 = stA.tile([128, CH_A], F32R, tag="pd1")
                nc.sync.dma_start(pd1[:], pdir_d[ci, 0:128].bitcast(F32R))
                pd2 = stA.tile([PF - 128, CH_A], F32R, tag="pd2")
                nc.sync.dma_start(pd2[:], pdir_d[ci, 128:PF].bitcast(F32R))

                ps = psA.tile([BL, CH_A], F32, tag="ps")
                for h0 in range(0, CH_A, 512):
                    hs = slice(h0, h0 + 512)
                    nc.tensor.matmul(
                        ps[:, hs], btT_r[:], sd[:, hs],
                        start=True, stop=False, skip_group_check=True,
                    )
                vs_sb = outA.tile([BL, CH_A], F32, tag="vs_sb")
                nc.scalar.copy(vs_sb[:], ps[:])
                nc.sync.dma_start(vshaped_d[ci], vs_sb[:])
                for h0 in range(0, CH_A, 512):
                    hs = slice(h0, h0 + 512)
                    nc.tensor.matmul(
                        ps[:, hs], poseT1[:], pd1[:, hs],
                        start=False, stop=False, skip_group_check=True,
                    )
                    nc.tensor.matmul(
                        ps[:, hs], poseT2[:], pd2[:, hs],
                        start=False, stop=True, skip_group_check=True,
                    )
                nc.scalar.copy(vp_buf[:, c0 : c0 + CH_A], ps[:])

            def do_b(ci):
                c0 = ci * CH_B
                verts_sb = outB.tile([BL, 3 * CH_B], F32, tag="verts")
                vv = verts_sb[:].rearrange("p (v m) -> p v m", m=3)
                for m in range(3):
                    ps_T = psB.tile([BL, 4, CH_B], F32, tag="T")
                    for n in range(4):
                        nc.tensor.matmul(
                            ps_T[:, n, :],
                            amnT[32 * n : 32 * n + J, m * BL : (m + 1) * BL],
                            wt_t[32 * n : 32 * n + J, c0 : c0 + CH_B],
                            start=True,
                            stop=True,
                            tile_position=(32 * n, 0),
                        )
                    p0 = tmpB.tile([BL, CH_B], F32, tag="p0")
                    nc.vector.tensor_mul(p0[:], ps_T[:, 0, :], vp3[:, c0 : c0 + CH_B, 0])
                    p1 = tmpB.tile([BL, CH_B], F32, tag="p1")
                    nc.vector.tensor_mul(p1[:], ps_T[:, 1, :], vp3[:, c0 : c0 + CH_B, 1])
                    p2 = tmpB.tile([BL, CH_B], F32, tag="p2")
                    nc.vector.tensor_mul(p2[:], ps_T[:, 2, :], vp3[:, c0 : c0 + CH_B, 2])
                    s0 = tmpB.tile([BL, CH_B], F32, tag="s0")
                    nc.gpsimd.tensor_add(s0[:], p0[:], p1[:])
                    nc.gpsimd.tensor_add(s0[:], s0[:], p2[:])
                    nc.vector.tensor_add(vv[:, :, m], ps_T[:, 3, :], s0[:])
                nc.sync.dma_start(verts_d[ci], verts_sb[:])

            a_next = 0
            for k in range(NB):
                need = min(NA, -(-3 * (k + 1) // 2) + 1)
                while a_next < need:
                    do_a(a_next)
                    a_next += 1
                do_b(k)
            while a_next < NA:
                do_a(a_next)
                a_next += 1

    nc.finalize()
    return nc


_NC_CACHE = {}


def _get_nc(debug=False):
    key = bool(debug)
    if key not in _NC_CACHE:
        _NC_CACHE[key] = build_nc(debug=debug)
    return _NC_CACHE[key]


def _host_prep(betas, full_pose, v_template, shapedirs, posedirs, J_regressor, lbs_weights):
    betas = np.asarray(betas, np.float32)
    full_pose = np.asarray(full_pose, np.float32)
    v_template = np.asarray(v_template, np.float32)
    shapedirs = np.asarray(shapedirs, np.float32)
    posedirs = np.asarray(posedirs, np.float32)
    J_regressor = np.asarray(J_regressor, np.float32)
    lbs_weights = np.asarray(lbs_weights, np.float32)

    sd_flat = shapedirs.reshape(VK, 10)
    vt_flat = v_template.reshape(VK)
    vt_hi = vt_flat.astype(np.dtype("bfloat16") if hasattr(np, "bfloat16") else np.float32)
    import ml_dtypes

    vt_hi = vt_flat.astype(ml_dtypes.bfloat16).astype(np.float32)
    vt_lo = vt_flat - vt_hi
    sdirT_aug = np.zeros((12, VKP), np.float32)
    sdirT_aug[0:10, 0:VK] = sd_flat.T
    sdirT_aug[10, 0:VK] = vt_hi
    sdirT_aug[11, 0:VK] = vt_lo
    sdirT_aug = np.ascontiguousarray(
        sdirT_aug.reshape(12, NA, CH_A).transpose(1, 0, 2)
    )  # [NA, 12, CH_A]
    jrs = np.einsum(
        "jv,vkl->ljk", J_regressor.astype(np.float64), shapedirs.astype(np.float64)
    ).reshape(10, 72)
    jt = (J_regressor.astype(np.float64) @ v_template[0].astype(np.float64)).reshape(1, 72)
    js2aug = np.ascontiguousarray(
        np.concatenate([jrs, jt, np.zeros((1, 72))], axis=0), np.float32
    )  # [12, 72]
    wT = np.zeros((128, VP), np.float32)
    for n in range(4):
        wT[32 * n : 32 * n + J, 0:V] = lbs_weights.T
    betas_aug = np.concatenate([betas, np.ones((B, 2), np.float32)], axis=1)  # [B, 12]
    ident = np.eye(128, dtype=np.float32)
    return betas_aug, full_pose, sdirT_aug, js2aug, wT, ident


def kernel(betas, full_pose, v_template, shapedirs, posedirs, J_regressor, lbs_weights, parents):
    betas_aug, full_pose, sdirT_aug, js2aug, wT, ident = _host_prep(
        betas, full_pose, v_template, shapedirs, posedirs, J_regressor, lbs_weights
    )
    pd = np.asarray(posedirs, np.float32)
    pdt = np.zeros((PF, VKP), np.float32)
    pdt[:, 0:VK] = pd
    posedirs = np.ascontiguousarray(pdt.reshape(PF, NA, CH_A).transpose(1, 0, 2))  # [NA, PF, CH_A]

    nc = _get_nc(debug=False)
    in_maps = []
    for i in range(NCORES):
        sl = slice(i * BL, (i + 1) * BL)
        in_maps.append(
            {
                "pose": np.ascontiguousarray(full_pose[sl]),
                "betas_aug": np.ascontiguousarray(betas_aug[sl]),
                "js2aug": js2aug,
                "sdirT_aug": sdirT_aug,
                "posedirs": posedirs,
                "wT": wT,
                "ident": ident,
            }
        )
    res = run_bass_kernel_spmd(nc, in_maps, list(range(NCORES)))

    verts = np.empty((B, V, 3), np.float32)
    v_shaped = np.empty((B, V, 3), np.float32)
    j_posed = np.empty((B, J, 3), np.float32)
    j_rest = np.empty((B, J, 3), np.float32)
    a_mats = np.empty((B, J, 4, 4), np.float32)
    for i, r in enumerate(res.results):
        sl = slice(i * BL, (i + 1) * BL)
        verts[sl] = (
            r["verts"].transpose(1, 0, 2).reshape(BL, VP, 3)[:, 0:V, :]
        )
        v_shaped[sl] = (
            r["v_shaped"].transpose(1, 0, 2).reshape(BL, VKP)[:, 0:VK].reshape(BL, V, 3)
        )
        j_posed[sl] = r["j_posed"].reshape(BL, J, 3)
        j_rest[sl] = r["j_rest"].reshape(BL, J, 3)
        a_mats[sl] = r["a_mats"].reshape(BL, J, 4, 4)
    return verts, j_posed, j_rest, a_mats, v_shaped


# revision 14
# speedup vs baseline: 1.3205x; 1.0892x over previous
"""SMPL body-model (shape/pose blendshapes + LBS) on 8 TRN2 NeuronCores.

Data-parallel over batch: 1024/8 = 128 rows per core, batch on SBUF
partitions everywhere. Heavy matmuls run in fp32r; the kinematic chain and
the per-vertex transform apply run on the vector engine in fp32.
"""

import sys

sys.path.insert(0, "/opt/trn_rl_repo")

import math
from contextlib import ExitStack

import numpy as np

import concourse.bass as bass
import concourse.tile as tile
from concourse import bacc, mybir
from concourse.bass_utils import run_bass_kernel_spmd

F32 = mybir.dt.float32
F32R = mybir.dt.float32r
ALU = mybir.AluOpType
AFT = mybir.ActivationFunctionType

B, V, J = 1024, 6890, 24
NCORES, BL = 8, 128
PF = (J - 1) * 9  # 207
VK = V * 3  # 20670
CH_A = 1024  # stage A chunk (columns of v*3, padded)
CH_B = 512  # stage B chunk (vertices, padded)
VP = 7168  # padded vertex count (14 * 512)
VKP = VP * 3  # 21504 = 21 * 1024
NA = VKP // CH_A  # 21 stage A chunks
NB = VP // CH_B  # 14 stage B chunks
SMPL_PARENTS = [-1, 0, 0, 0, 1, 2, 3, 4, 5, 6, 7, 8, 9, 9, 9, 12, 13, 14, 16, 17, 18, 19, 20, 21]

# (j0, nj, p0, pstep) groups with affine parent indexing, grouped by tree depth
CHAIN_GROUPS = [
    (1, 3, 0, 0),
    (4, 3, 1, 1),
    (7, 3, 4, 1),
    (10, 3, 7, 1),
    (13, 2, 9, 0),
    (15, 3, 12, 1),
    (18, 2, 16, 1),
    (20, 2, 18, 1),
    (22, 2, 20, 1),
]
# rel_joints only needs J_rest, so consecutive-parent runs can span depths
REL_GROUPS = [(1, 3, 0, 0), (4, 9, 1, 1), (13, 2, 9, 0), (15, 3, 12, 1), (18, 6, 16, 1)]


def _jsl(view, j0, nj, step):
    """view[:, j0 : j0+nj] with parent step 1 or broadcast (step 0)."""
    if step == 1:
        return view[:, j0 : j0 + nj]
    sl = view[:, j0 : j0 + 1]
    return sl.broadcast_to((sl.shape[0], nj) + tuple(sl.shape[2:]))


def build_nc(debug=False):
    nc = bacc.Bacc("TRN2", target_bir_lowering=False, debug=False, num_devices=NCORES)

    pose_d = nc.dram_tensor("pose", [BL, 72], F32, kind="ExternalInput")
    ba_d = nc.dram_tensor("betas_aug", [BL, 12], F32, kind="ExternalInput")
    js2_d = nc.dram_tensor("js2aug", [12, 72], F32, kind="ExternalInput")
    sdir_d = nc.dram_tensor("sdirT_aug", [NA, 12, CH_A], F32, kind="ExternalInput")
    pdir_d = nc.dram_tensor("posedirs", [NA, PF, CH_A], F32, kind="ExternalInput")
    wt_d = nc.dram_tensor("wT", [128, VP], F32, kind="ExternalInput")
    id_d = nc.dram_tensor("ident", [128, 128], F32, kind="ExternalInput")

    verts_d = nc.dram_tensor("verts", [NB, BL, 3 * CH_B], F32, kind="ExternalOutput")
    vshaped_d = nc.dram_tensor("v_shaped", [NA, BL, CH_A], F32, kind="ExternalOutput")
    jposed_d = nc.dram_tensor("j_posed", [BL, 72], F32, kind="ExternalOutput")
    jrest_d = nc.dram_tensor("j_rest", [BL, 72], F32, kind="ExternalOutput")
    a_d = nc.dram_tensor("a_mats", [BL, 384], F32, kind="ExternalOutput")
    if debug:
        rot_dbg = nc.dram_tensor("rot_dbg", [BL, 216], F32, kind="ExternalOutput")
        rg_dbg = nc.dram_tensor("rg_dbg", [BL, 216], F32, kind="ExternalOutput")

    with tile.TileContext(nc) as tc, ExitStack() as ctx:
        keep = ctx.enter_context(tc.tile_pool(name="keep", bufs=1))
        sm = ctx.enter_context(tc.tile_pool(name="small", bufs=1))

        # ---------------- loads ----------------
        pose_t = sm.tile([BL, 72], F32)
        nc.sync.dma_start(pose_t[:], pose_d[:])
        ba_t = sm.tile([BL, 12], F32)
        nc.sync.dma_start(ba_t[:], ba_d[:])
        id_t = keep.tile([128, 128], F32)
        nc.sync.dma_start(id_t[:], id_d[:])
        js2_t = sm.tile([12, 72], F32)
        nc.sync.dma_start(js2_t[:], js2_d[:])
        wt_t = keep.tile([128, VP], F32R)
        nc.sync.dma_start(wt_t[:], wt_d[:].bitcast(F32R))

        vp_buf = keep.tile([BL, VKP], F32)  # v_posed, resident (padded)
        btT_r = keep.tile([12, BL], F32R)
        poseT1 = keep.tile([128, BL], F32R)
        poseT2 = keep.tile([PF - 128, BL], F32R)
        amnT = keep.tile([128, 3 * BL], F32R)  # per m: 4 row-packed [24,128] lhsT

        with tc.tile_pool(name="ps0", bufs=1, space="PSUM") as ps0:
            # betas^T (for J_rest fp32 matmul and stage A fp32r lhsT)
            ps_bt = ps0.tile([12, BL], F32)
            nc.tensor.transpose(ps_bt[:], ba_t[:], id_t[:])
            btT_f = sm.tile([12, BL], F32)
            nc.scalar.copy(btT_f[:], ps_bt[:])
            nc.vector.tensor_copy(btT_r[:], ps_bt[:])

            # J_rest = betas_aug @ js2aug  (exact fp32)
            ps_jr = ps0.tile([BL, 72], F32)
            nc.tensor.matmul(ps_jr[:], btT_f[:], js2_t[:], start=True, stop=True)
            jrest_t = sm.tile([BL, 72], F32)
            nc.scalar.copy(jrest_t[:], ps_jr[:])
            nc.sync.dma_start(jrest_d[:], jrest_t[:])

            # ---------------- Rodrigues ----------------
            rv8 = sm.tile([BL, 72], F32)
            nc.vector.tensor_scalar_add(rv8[:], pose_t[:], 1e-8)
            sq = sm.tile([BL, 72], F32)
            nc.vector.tensor_mul(sq[:], rv8[:], rv8[:])
            n2 = sm.tile([BL, J], F32)
            nc.vector.reduce_sum(
                n2[:], sq[:].rearrange("p (j k) -> p j k", k=3), axis=mybir.AxisListType.X
            )
            ang = sm.tile([BL, J], F32)
            nc.scalar.activation(ang[:], n2[:], AFT.Sqrt)
            inv = sm.tile([BL, J], F32)
            nc.vector.reciprocal(inv[:], ang[:])
            s_t = sm.tile([BL, J], F32)
            nc.scalar.activation(s_t[:], ang[:], AFT.Sin)
            c_t = sm.tile([BL, J], F32)
            halfpi = sm.tile([BL, 1], F32)
            nc.vector.memset(halfpi[:], math.pi / 2)
            nc.scalar.activation(c_t[:], ang[:], AFT.Sin, bias=halfpi[:])

            axis_t = sm.tile([BL, 72], F32)  # [b, (j,3)]
            ax3 = axis_t[:].rearrange("p (j k) -> p j k", k=3)
            nc.vector.tensor_mul(
                ax3,
                pose_t[:].rearrange("p (j k) -> p j k", k=3),
                inv[:].unsqueeze(2).broadcast_to([BL, J, 3]),
            )

            rot = sm.tile([BL, 216], F32)  # [b, (j,m,n)] local rotations
            r4 = rot[:].rearrange("p (j m n) -> p j m n", m=3, n=3)
            # (1-c) * outer(axis, axis)
            omc = sm.tile([BL, J], F32)
            nc.vector.tensor_scalar(omc[:], c_t[:], -1.0, 1.0, op0=ALU.mult, op1=ALU.add)
            nc.vector.tensor_mul(
                r4,
                ax3.unsqueeze(3).broadcast_to([BL, J, 3, 3]),
                ax3.unsqueeze(2).broadcast_to([BL, J, 3, 3]),
            )
            r9 = rot[:].rearrange("p (j a) -> p j a", a=9)
            nc.vector.tensor_mul(
                r9, r9, omc[:].unsqueeze(2).broadcast_to([BL, J, 9])
            )
            # + c on the diagonal
            nc.vector.tensor_add(
                r9[:, :, 0:9:4], r9[:, :, 0:9:4], c_t[:].unsqueeze(2).broadcast_to([BL, J, 3])
            )
            # +/- s*axis off-diagonals
            sa = sm.tile([BL, 72], F32)
            sa3 = sa[:].rearrange("p (j k) -> p j k", k=3)
            nc.vector.tensor_mul(sa3, ax3, s_t[:].unsqueeze(2).broadcast_to([BL, J, 3]))
            for pos, comp, sign in ((1, 2, -1), (3, 2, 1), (2, 1, 1), (6, 1, -1), (5, 0, -1), (7, 0, 1)):
                op = nc.vector.tensor_add if sign > 0 else nc.vector.tensor_sub
                op(r9[:, :, pos], r9[:, :, pos], sa3[:, :, comp])
            if debug:
                rot_sb = sm.tile([BL, 216], F32)
                nc.vector.tensor_copy(rot_sb[:], rot[:])
                nc.sync.dma_start(rot_dbg[:], rot_sb[:])

            # ---------------- rel joints ----------------
            rel = sm.tile([BL, 72], F32)
            jr3 = jrest_t[:].rearrange("p (j k) -> p j k", k=3)
            rl3 = rel[:].rearrange("p (j k) -> p j k", k=3)
            nc.vector.tensor_copy(rl3[:, 0:1], jr3[:, 0:1])
            for j0, nj, p0, pstep in REL_GROUPS:
                nc.vector.tensor_sub(
                    rl3[:, j0 : j0 + nj], jr3[:, j0 : j0 + nj], _jsl(jr3, p0, nj, pstep)
                )

            # ---------------- kinematic chain ----------------
            rg = sm.tile([BL, 216], F32)
            tg = sm.tile([BL, 72], F32)
            g4 = rg[:].rearrange("p (j m n) -> p j m n", m=3, n=3)
            t3 = tg[:].rearrange("p (j k) -> p j k", k=3)
            nc.vector.tensor_copy(rg[:, 0:9], rot[:, 0:9])
            nc.vector.tensor_copy(t3[:, 0:1], rl3[:, 0:1])
            tmpR = sm.tile([BL, 27], F32)
            tmpt = sm.tile([BL, 9], F32)
            for j0, nj, p0, pstep in CHAIN_GROUPS:
                dstR = g4[:, j0 : j0 + nj]  # [b, nj, m, n]
                locR = r4[:, j0 : j0 + nj]
                parR = _jsl(g4, p0, nj, pstep)
                tR = tmpR[:].rearrange("p (j m n) -> p j m n", m=3, n=3)[:, 0:nj]
                for k in range(3):
                    a_in = parR[:, :, :, k].unsqueeze(3).broadcast_to([BL, nj, 3, 3])
                    b_in = locR[:, :, k, :].unsqueeze(2).broadcast_to([BL, nj, 3, 3])
                    if k == 0:
                        nc.vector.tensor_mul(dstR, a_in, b_in)
                    else:
                        nc.vector.tensor_mul(tR, a_in, b_in)
                        nc.vector.tensor_add(dstR, dstR, tR)
                # translations
                dstT = t3[:, j0 : j0 + nj]
                locT = rl3[:, j0 : j0 + nj]
                tT = tmpt[:].rearrange("p (j k) -> p j k", k=3)[:, 0:nj]
                for k in range(3):
                    a_in = parR[:, :, :, k]  # [b, nj, 3(m)]
                    b_in = locT[:, :, k].unsqueeze(2).broadcast_to([BL, nj, 3])
                    if k == 0:
                        nc.vector.tensor_mul(dstT, a_in, b_in)
                    else:
                        nc.vector.tensor_mul(tT, a_in, b_in)
                        nc.vector.tensor_add(dstT, dstT, tT)
                nc.vector.tensor_add(dstT, dstT, _jsl(t3, p0, nj, pstep))

            nc.sync.dma_start(jposed_d[:], tg[:])
            if debug:
                rg_sb = sm.tile([BL, 216], F32)
                nc.vector.tensor_copy(rg_sb[:], rg[:])
                nc.sync.dma_start(rg_dbg[:], rg_sb[:])

            # ---------------- A matrices ----------------
            # ta = tg - sum_k Rg[:,:, :,k] * J_rest[:,:,k]
            ta = sm.tile([BL, 72], F32)
            ta3 = ta[:].rearrange("p (j k) -> p j k", k=3)
            acc = sm.tile([BL, 72], F32)
            acc3 = acc[:].rearrange("p (j k) -> p j k", k=3)
            for k in range(3):
                a_in = g4[:, :, :, k]  # [b, J, 3(m)]
                b_in = jr3[:, :, k].unsqueeze(2).broadcast_to([BL, J, 3])
                if k == 0:
                    nc.vector.tensor_mul(acc3, a_in, b_in)
                else:
                    nc.vector.tensor_mul(ta3, a_in, b_in)
                    nc.vector.tensor_add(acc3, acc3, ta3)
            nc.vector.tensor_sub(ta3, t3, acc3)

            a_full = sm.tile([BL, 384], F32)
            a4 = a_full[:].rearrange("p (j m n) -> p j m n", m=4, n=4)
            nc.vector.memset(a_full[:], 0.0)
            nc.vector.memset(a4[:, :, 3, 3], 1.0)
            nc.vector.tensor_copy(a4[:, :, 0:3, 0:3], g4)
            nc.vector.tensor_copy(a4[:, :, 0:3, 3], ta3)
            nc.sync.dma_start(a_d[:], a_full[:])

            # ---------------- pose_feature^T ----------------
            nc.vector.tensor_scalar_add(
                rot[:, 9:216].rearrange("p (j a) -> p j a", a=9)[:, :, 0:9:4],
                rot[:, 9:216].rearrange("p (j a) -> p j a", a=9)[:, :, 0:9:4],
                -1.0,
            )
            ps_p1 = ps0.tile([128, BL], F32)
            nc.tensor.transpose(ps_p1[:], rot[:, 9:137], id_t[:])
            nc.vector.tensor_copy(poseT1[:], ps_p1[:])
            ps_p2 = ps0.tile([PF - 128, BL], F32)
            nc.tensor.transpose(ps_p2[:], rot[:, 137:216], id_t[:])
            nc.vector.tensor_copy(poseT2[:], ps_p2[:])

        # ---------------- A^T slices for LBS ----------------
        with (
            tc.tile_pool(name="psT", bufs=3, space="PSUM") as psT,
            tc.tile_pool(name="trT", bufs=3) as trT,
        ):
            for m in range(3):
                for n in range(4):
                    ps_t = psT.tile([J, BL], F32, tag="ps_t")
                    nc.tensor.transpose(ps_t[:], a4[:, :, m, n], id_t[:])
                    tr_t = trT.tile([J, BL], F32R, tag="tr_t")
                    nc.scalar.copy(tr_t[:], ps_t[:])
                    nc.sync.dma_start(
                        amnT[32 * n : 32 * n + J, m * BL : (m + 1) * BL], tr_t[:]
                    )

        # ---------------- stages A (blendshapes) + B (LBS), interleaved ----------------
        vp3 = vp_buf[:].rearrange("p (v k) -> p v k", k=3)
        with (
            tc.tile_pool(name="streamA", bufs=3) as stA,
            tc.tile_pool(name="outA", bufs=2) as outA,
            tc.tile_pool(name="psA", bufs=2, space="PSUM") as psA,
            tc.tile_pool(name="outB", bufs=3) as outB,
            tc.tile_pool(name="tmpB", bufs=2) as tmpB,
            tc.tile_pool(name="psB", bufs=1, space="PSUM") as psB,
        ):
            a_state = {}

            def a_load(ci):
                sd = stA.tile([12, CH_A], F32R, tag="sd")
                nc.sync.dma_start(sd[:], sdir_d[ci].bitcast(F32R))
                pd1 = stA.tile([128, CH_A], F32R, tag="pd1")
                nc.sync.dma_start(pd1[:], pdir_d[ci, 0:128].bitcast(F32R))
                pd2 = stA.tile([PF - 128, CH_A], F32R, tag="pd2")
                nc.sync.dma_start(pd2[:], pdir_d[ci, 128:PF].bitcast(F32R))
                ps = psA.tile([BL, CH_A], F32, tag="ps")
                a_state[ci] = (sd, pd1, pd2, ps)

            def a_bet(ci):
                sd, pd1, pd2, ps = a_state[ci]
                for h0 in (0, 512):
                    hs = slice(h0, h0 + 512)
                    nc.tensor.matmul(
                        ps[:, hs], btT_r[:], sd[:, hs],
                        start=True, stop=False, skip_group_check=True,
                    )

            def a_vs(ci):
                _, _, _, ps = a_state[ci]
                vs_sb = outA.tile([BL, CH_A], F32, tag="vs_sb")
                nc.scalar.copy(vs_sb[:], ps[:])
                nc.sync.dma_start(vshaped_d[ci], vs_sb[:])

            def a_pose(ci, h0):
                sd, pd1, pd2, ps = a_state[ci]
                hs = slice(h0, h0 + 512)
                nc.tensor.matmul(
                    ps[:, hs], poseT1[:], pd1[:, hs],
                    start=False, stop=False, skip_group_check=True,
                )
                nc.tensor.matmul(
                    ps[:, hs], poseT2[:], pd2[:, hs],
                    start=False, stop=True, skip_group_check=True,
                )

            def a_fin(ci):
                _, _, _, ps = a_state.pop(ci)
                c0 = ci * CH_A
                nc.scalar.copy(vp_buf[:, c0 : c0 + CH_A], ps[:])

            b_state = {}

            def b_round(ci, m):
                c0 = ci * CH_B
                if m == 0:
                    verts_sb = outB.tile([BL, 3 * CH_B], F32, tag="verts")
                    b_state[ci] = verts_sb
                verts_sb = b_state[ci]
                vv = verts_sb[:].rearrange("p (v m) -> p v m", m=3)
                ps_T = psB.tile([BL, 4, CH_B], F32, tag="T")
                for n in range(4):
                    nc.tensor.matmul(
                        ps_T[:, n, :],
                        amnT[32 * n : 32 * n + J, m * BL : (m + 1) * BL],
                        wt_t[32 * n : 32 * n + J, c0 : c0 + CH_B],
                        start=True,
                        stop=True,
                        tile_position=(32 * n, 0),
                    )
                # consume PSUM early (all PSUM reads on DVE, right after the matmuls)
                p01 = tmpB.tile([BL, 2, CH_B], F32, tag="p01")
                vpA = vp_buf[:][
                    :, 3 * c0 : 3 * (c0 + CH_B)
                ].rearrange("p (v k) -> p v k", k=3)
                in1 = vpA[:, :, 0:2].transpose([0, 2, 1])
                nc.vector.tensor_mul(p01[:], ps_T[:, 0:2, :], in1)
                p2 = tmpB.tile([BL, CH_B], F32, tag="p2")
                nc.vector.tensor_mul(p2[:], ps_T[:, 2, :], vp3[:, c0 : c0 + CH_B, 2])
                s2 = tmpB.tile([BL, CH_B], F32, tag="s2")
                nc.vector.tensor_add(s2[:], ps_T[:, 3, :], p2[:])
                s1 = tmpB.tile([BL, CH_B], F32, tag="s1")
                nc.gpsimd.tensor_add(s1[:], p01[:, 0, :], p01[:, 1, :])
                nc.gpsimd.tensor_add(vv[:, :, m], s1[:], s2[:])
                if m == 2:
                    nc.sync.dma_start(verts_d[ci], verts_sb[:])
                    del b_state[ci]

            # ---- schedule: A-halves and B-m-rounds interleaved
            a_steps = []
            for ci in range(NA):
                a_steps.append(("load", ci))
                a_steps.append(("bet", ci))
                a_steps.append(("vs", ci))
                a_steps.append(("pose", ci, 0))
                a_steps.append(("pose", ci, 512))
                a_steps.append(("fin", ci))
            a_ptr = 0
            a_done = 0

            def emit_a():
                nonlocal_ns = {}
                return None

            def emit_next_a():
                global _unused
                return None

            def run_a_step():
                nonlocal a_ptr, a_done
                if a_ptr >= len(a_steps):
                    return False
                step = a_steps[a_ptr]
                a_ptr += 1
                if step[0] == "load":
                    a_load(step[1])
                elif step[0] == "bet":
                    a_bet(step[1])
                elif step[0] == "vs":
                    a_vs(step[1])
                elif step[0] == "pose":
                    a_pose(step[1], step[2])
                else:
                    a_fin(step[1])
                    a_done += 1
                return True

            for k in range(NB):
                need = min(NA, -(-3 * (k + 1) // 2) + 1)
                while a_done < need:
                    run_a_step()
                for m in range(3):
                    b_round(k, m)
                    run_a_step()
            while run_a_step():
                pass

    nc.finalize()
    return nc


_NC_CACHE = {}


def _get_nc(debug=False):
    key = bool(debug)
    if key not in _NC_CACHE:
        _NC_CACHE[key] = build_nc(debug=debug)
    return _NC_CACHE[key]


def _host_prep(betas, full_pose, v_template, shapedirs, posedirs, J_regressor, lbs_weights):
    betas = np.asarray(betas, np.float32)
    full_pose = np.asarray(full_pose, np.float32)
    v_template = np.asarray(v_template, np.float32)
    shapedirs = np.asarray(shapedirs, np.float32)
    posedirs = np.asarray(posedirs, np.float32)
    J_regressor = np.asarray(J_regressor, np.float32)
    lbs_weights = np.asarray(lbs_weights, np.float32)

    sd_flat = shapedirs.reshape(VK, 10)
    vt_flat = v_template.reshape(VK)
    vt_hi = vt_flat.astype(np.dtype("bfloat16") if hasattr(np, "bfloat16") else np.float32)
    import ml_dtypes

    vt_hi = vt_flat.astype(ml_dtypes.bfloat16).astype(np.float32)
    vt_lo = vt_flat - vt_hi
    sdirT_aug = np.zeros((12, VKP), np.float32)
    sdirT_aug[0:10, 0:VK] = sd_flat.T
    sdirT_aug[10, 0:VK] = vt_hi
    sdirT_aug[11, 0:VK] = vt_lo
    sdirT_aug = np.ascontiguousarray(
        sdirT_aug.reshape(12, NA, CH_A).transpose(1, 0, 2)
    )  # [NA, 12, CH_A]
    jrs = np.einsum(
        "jv,vkl->ljk", J_regressor.astype(np.float64), shapedirs.astype(np.float64)
    ).reshape(10, 72)
    jt = (J_regressor.astype(np.float64) @ v_template[0].astype(np.float64)).reshape(1, 72)
    js2aug = np.ascontiguousarray(
        np.concatenate([jrs, jt, np.zeros((1, 72))], axis=0), np.float32
    )  # [12, 72]
    wT = np.zeros((128, VP), np.float32)
    for n in range(4):
        wT[32 * n : 32 * n + J, 0:V] = lbs_weights.T
    betas_aug = np.concatenate([betas, np.ones((B, 2), np.float32)], axis=1)  # [B, 12]
    ident = np.eye(128, dtype=np.float32)
    return betas_aug, full_pose, sdirT_aug, js2aug, wT, ident


def kernel(betas, full_pose, v_template, shapedirs, posedirs, J_regressor, lbs_weights, parents):
    betas_aug, full_pose, sdirT_aug, js2aug, wT, ident = _host_prep(
        betas, full_pose, v_template, shapedirs, posedirs, J_regressor, lbs_weights
    )
    pd = np.asarray(posedirs, np.float32)
    pdt = np.zeros((PF, VKP), np.float32)
    pdt[:, 0:VK] = pd
    posedirs = np.ascontiguousarray(pdt.reshape(PF, NA, CH_A).transpose(1, 0, 2))  # [NA, PF, CH_A]

    nc = _get_nc(debug=False)
    in_maps = []
    for i in range(NCORES):
        sl = slice(i * BL, (i + 1) * BL)
        in_maps.append(
            {
                "pose": np.ascontiguousarray(full_pose[sl]),
                "betas_aug": np.ascontiguousarray(betas_aug[sl]),
                "js2aug": js2aug,
                "sdirT_aug": sdirT_aug,
                "posedirs": posedirs,
                "wT": wT,
                "ident": ident,
            }
        )
    res = run_bass_kernel_spmd(nc, in_maps, list(range(NCORES)))

    verts = np.empty((B, V, 3), np.float32)
    v_shaped = np.empty((B, V, 3), np.float32)
    j_posed = np.empty((B, J, 3), np.float32)
    j_rest = np.empty((B, J, 3), np.float32)
    a_mats = np.empty((B, J, 4, 4), np.float32)
    for i, r in enumerate(res.results):
        sl = slice(i * BL, (i + 1) * BL)
        verts[sl] = (
            r["verts"].transpose(1, 0, 2).reshape(BL, VP, 3)[:, 0:V, :]
        )
        v_shaped[sl] = (
            r["v_shaped"].transpose(1, 0, 2).reshape(BL, VKP)[:, 0:VK].reshape(BL, V, 3)
        )
        j_posed[sl] = r["j_posed"].reshape(BL, J, 3)
        j_rest[sl] = r["j_rest"].reshape(BL, J, 3)
        a_mats[sl] = r["a_mats"].reshape(BL, J, 4, 4)
    return verts, j_posed, j_rest, a_mats, v_shaped


# revision 15
# speedup vs baseline: 1.7293x; 1.3096x over previous
"""SMPL body-model (shape/pose blendshapes + LBS) on 8 TRN2 NeuronCores.

Data-parallel over batch: 1024/8 = 128 rows per core, batch on SBUF
partitions everywhere. Heavy matmuls run in fp32r; the kinematic chain and
the per-vertex transform apply run on the vector engine in fp32.
"""

import sys

sys.path.insert(0, "/opt/trn_rl_repo")

import math
from contextlib import ExitStack

import numpy as np

import concourse.bass as bass
import concourse.tile as tile
from concourse import bacc, mybir
from concourse.bass_utils import run_bass_kernel_spmd

F32 = mybir.dt.float32
F32R = mybir.dt.float32r
F16 = mybir.dt.float16
ALU = mybir.AluOpType
AFT = mybir.ActivationFunctionType

B, V, J = 1024, 6890, 24
NCORES, BL = 8, 128
PF = (J - 1) * 9  # 207
VK = V * 3  # 20670
CH_A = 1024  # stage A chunk (columns of v*3, padded)
CH_B = 512  # stage B chunk (vertices, padded)
VP = 7168  # padded vertex count (14 * 512)
VKP = VP * 3  # 21504 = 21 * 1024
NA = VKP // CH_A  # 21 stage A chunks
NB = VP // CH_B  # 14 stage B chunks
SMPL_PARENTS = [-1, 0, 0, 0, 1, 2, 3, 4, 5, 6, 7, 8, 9, 9, 9, 12, 13, 14, 16, 17, 18, 19, 20, 21]

# (j0, nj, p0, pstep) groups with affine parent indexing, grouped by tree depth
CHAIN_GROUPS = [
    (1, 3, 0, 0),
    (4, 3, 1, 1),
    (7, 3, 4, 1),
    (10, 3, 7, 1),
    (13, 2, 9, 0),
    (15, 3, 12, 1),
    (18, 2, 16, 1),
    (20, 2, 18, 1),
    (22, 2, 20, 1),
]
# rel_joints only needs J_rest, so consecutive-parent runs can span depths
REL_GROUPS = [(1, 3, 0, 0), (4, 9, 1, 1), (13, 2, 9, 0), (15, 3, 12, 1), (18, 6, 16, 1)]


def _jsl(view, j0, nj, step):
    """view[:, j0 : j0+nj] with parent step 1 or broadcast (step 0)."""
    if step == 1:
        return view[:, j0 : j0 + nj]
    sl = view[:, j0 : j0 + 1]
    return sl.broadcast_to((sl.shape[0], nj) + tuple(sl.shape[2:]))


def build_nc(debug=False):
    nc = bacc.Bacc("TRN2", target_bir_lowering=False, debug=False, num_devices=NCORES)

    pose_d = nc.dram_tensor("pose", [BL, 72], F32, kind="ExternalInput")
    ba_d = nc.dram_tensor("betas_aug", [BL, 12], F32, kind="ExternalInput")
    js2_d = nc.dram_tensor("js2aug", [12, 72], F32, kind="ExternalInput")
    sdir_d = nc.dram_tensor("sdirT_aug", [NA, 12, CH_A], F16, kind="ExternalInput")
    pdir_d = nc.dram_tensor("posedirs", [NA, PF, CH_A], F16, kind="ExternalInput")
    wt_d = nc.dram_tensor("wT", [128, VP], F32, kind="ExternalInput")
    id_d = nc.dram_tensor("ident", [128, 128], F32, kind="ExternalInput")

    verts_d = nc.dram_tensor("verts", [NB, BL, 3 * CH_B], F32, kind="ExternalOutput")
    vshaped_d = nc.dram_tensor("v_shaped", [NA, BL, CH_A], F32, kind="ExternalOutput")
    jposed_d = nc.dram_tensor("j_posed", [BL, 72], F32, kind="ExternalOutput")
    jrest_d = nc.dram_tensor("j_rest", [BL, 72], F32, kind="ExternalOutput")
    a_d = nc.dram_tensor("a_mats", [BL, 384], F32, kind="ExternalOutput")
    if debug:
        rot_dbg = nc.dram_tensor("rot_dbg", [BL, 216], F32, kind="ExternalOutput")
        rg_dbg = nc.dram_tensor("rg_dbg", [BL, 216], F32, kind="ExternalOutput")

    with tile.TileContext(nc) as tc, ExitStack() as ctx:
        keep = ctx.enter_context(tc.tile_pool(name="keep", bufs=1))
        sm = ctx.enter_context(tc.tile_pool(name="small", bufs=1))

        # ---------------- loads ----------------
        pose_t = sm.tile([BL, 72], F32)
        nc.sync.dma_start(pose_t[:], pose_d[:])
        ba_t = sm.tile([BL, 12], F32)
        nc.sync.dma_start(ba_t[:], ba_d[:])
        id_t = keep.tile([128, 128], F32)
        nc.sync.dma_start(id_t[:], id_d[:])
        js2_t = sm.tile([12, 72], F32)
        nc.sync.dma_start(js2_t[:], js2_d[:])
        wt_t = keep.tile([128, VP], F32R)
        nc.sync.dma_start(wt_t[:], wt_d[:].bitcast(F32R))

        vp_buf = keep.tile([BL, VKP], F32)  # v_posed, resident (padded)
        btT_r = keep.tile([12, BL], F16)
        poseT1 = keep.tile([128, BL], F16)
        poseT2 = keep.tile([PF - 128, BL], F16)
        amnT = keep.tile([128, 3 * BL], F32R)  # per m: 4 row-packed [24,128] lhsT

        with tc.tile_pool(name="ps0", bufs=1, space="PSUM") as ps0:
            # betas^T (for J_rest fp32 matmul and stage A fp32r lhsT)
            ps_bt = ps0.tile([12, BL], F32)
            nc.tensor.transpose(ps_bt[:], ba_t[:], id_t[:])
            btT_f = sm.tile([12, BL], F32)
            nc.scalar.copy(btT_f[:], ps_bt[:])
            nc.vector.tensor_copy(btT_r[:], ps_bt[:])

            # J_rest = betas_aug @ js2aug  (exact fp32)
            ps_jr = ps0.tile([BL, 72], F32)
            nc.tensor.matmul(ps_jr[:], btT_f[:], js2_t[:], start=True, stop=True)
            jrest_t = sm.tile([BL, 72], F32)
            nc.scalar.copy(jrest_t[:], ps_jr[:])
            nc.sync.dma_start(jrest_d[:], jrest_t[:])

            # ---------------- Rodrigues ----------------
            rv8 = sm.tile([BL, 72], F32)
            nc.vector.tensor_scalar_add(rv8[:], pose_t[:], 1e-8)
            sq = sm.tile([BL, 72], F32)
            nc.vector.tensor_mul(sq[:], rv8[:], rv8[:])
            n2 = sm.tile([BL, J], F32)
            nc.vector.reduce_sum(
                n2[:], sq[:].rearrange("p (j k) -> p j k", k=3), axis=mybir.AxisListType.X
            )
            ang = sm.tile([BL, J], F32)
            nc.scalar.activation(ang[:], n2[:], AFT.Sqrt)
            inv = sm.tile([BL, J], F32)
            nc.vector.reciprocal(inv[:], ang[:])
            s_t = sm.tile([BL, J], F32)
            nc.scalar.activation(s_t[:], ang[:], AFT.Sin)
            c_t = sm.tile([BL, J], F32)
            halfpi = sm.tile([BL, 1], F32)
            nc.vector.memset(halfpi[:], math.pi / 2)
            nc.scalar.activation(c_t[:], ang[:], AFT.Sin, bias=halfpi[:])

            axis_t = sm.tile([BL, 72], F32)  # [b, (j,3)]
            ax3 = axis_t[:].rearrange("p (j k) -> p j k", k=3)
            nc.vector.tensor_mul(
                ax3,
                pose_t[:].rearrange("p (j k) -> p j k", k=3),
                inv[:].unsqueeze(2).broadcast_to([BL, J, 3]),
            )

            rot = sm.tile([BL, 216], F32)  # [b, (j,m,n)] local rotations
            r4 = rot[:].rearrange("p (j m n) -> p j m n", m=3, n=3)
            # (1-c) * outer(axis, axis)
            omc = sm.tile([BL, J], F32)
            nc.vector.tensor_scalar(omc[:], c_t[:], -1.0, 1.0, op0=ALU.mult, op1=ALU.add)
            nc.vector.tensor_mul(
                r4,
                ax3.unsqueeze(3).broadcast_to([BL, J, 3, 3]),
                ax3.unsqueeze(2).broadcast_to([BL, J, 3, 3]),
            )
            r9 = rot[:].rearrange("p (j a) -> p j a", a=9)
            nc.vector.tensor_mul(
                r9, r9, omc[:].unsqueeze(2).broadcast_to([BL, J, 9])
            )
            # + c on the diagonal
            nc.vector.tensor_add(
                r9[:, :, 0:9:4], r9[:, :, 0:9:4], c_t[:].unsqueeze(2).broadcast_to([BL, J, 3])
            )
            # +/- s*axis off-diagonals
            sa = sm.tile([BL, 72], F32)
            sa3 = sa[:].rearrange("p (j k) -> p j k", k=3)
            nc.vector.tensor_mul(sa3, ax3, s_t[:].unsqueeze(2).broadcast_to([BL, J, 3]))
            for pos, comp, sign in ((1, 2, -1), (3, 2, 1), (2, 1, 1), (6, 1, -1), (5, 0, -1), (7, 0, 1)):
                op = nc.vector.tensor_add if sign > 0 else nc.vector.tensor_sub
                op(r9[:, :, pos], r9[:, :, pos], sa3[:, :, comp])
            if debug:
                rot_sb = sm.tile([BL, 216], F32)
                nc.vector.tensor_copy(rot_sb[:], rot[:])
                nc.sync.dma_start(rot_dbg[:], rot_sb[:])

            # ---------------- rel joints ----------------
            rel = sm.tile([BL, 72], F32)
            jr3 = jrest_t[:].rearrange("p (j k) -> p j k", k=3)
            rl3 = rel[:].rearrange("p (j k) -> p j k", k=3)
            nc.vector.tensor_copy(rl3[:, 0:1], jr3[:, 0:1])
            for j0, nj, p0, pstep in REL_GROUPS:
                nc.vector.tensor_sub(
                    rl3[:, j0 : j0 + nj], jr3[:, j0 : j0 + nj], _jsl(jr3, p0, nj, pstep)
                )

            # ---------------- kinematic chain ----------------
            rg = sm.tile([BL, 216], F32)
            tg = sm.tile([BL, 72], F32)
            g4 = rg[:].rearrange("p (j m n) -> p j m n", m=3, n=3)
            t3 = tg[:].rearrange("p (j k) -> p j k", k=3)
            nc.vector.tensor_copy(rg[:, 0:9], rot[:, 0:9])
            nc.vector.tensor_copy(t3[:, 0:1], rl3[:, 0:1])
            tmpR = sm.tile([BL, 27], F32)
            tmpt = sm.tile([BL, 9], F32)
            for j0, nj, p0, pstep in CHAIN_GROUPS:
                dstR = g4[:, j0 : j0 + nj]  # [b, nj, m, n]
                locR = r4[:, j0 : j0 + nj]
                parR = _jsl(g4, p0, nj, pstep)
                tR = tmpR[:].rearrange("p (j m n) -> p j m n", m=3, n=3)[:, 0:nj]
                for k in range(3):
                    a_in = parR[:, :, :, k].unsqueeze(3).broadcast_to([BL, nj, 3, 3])
                    b_in = locR[:, :, k, :].unsqueeze(2).broadcast_to([BL, nj, 3, 3])
                    if k == 0:
                        nc.vector.tensor_mul(dstR, a_in, b_in)
                    else:
                        nc.vector.tensor_mul(tR, a_in, b_in)
                        nc.vector.tensor_add(dstR, dstR, tR)
                # translations
                dstT = t3[:, j0 : j0 + nj]
                locT = rl3[:, j0 : j0 + nj]
                tT = tmpt[:].rearrange("p (j k) -> p j k", k=3)[:, 0:nj]
                for k in range(3):
                    a_in = parR[:, :, :, k]  # [b, nj, 3(m)]
                    b_in = locT[:, :, k].unsqueeze(2).broadcast_to([BL, nj, 3])
                    if k == 0:
                        nc.vector.tensor_mul(dstT, a_in, b_in)
                    else:
                        nc.vector.tensor_mul(tT, a_in, b_in)
                        nc.vector.tensor_add(dstT, dstT, tT)
                nc.vector.tensor_add(dstT, dstT, _jsl(t3, p0, nj, pstep))

            nc.sync.dma_start(jposed_d[:], tg[:])
            if debug:
                rg_sb = sm.tile([BL, 216], F32)
                nc.vector.tensor_copy(rg_sb[:], rg[:])
                nc.sync.dma_start(rg_dbg[:], rg_sb[:])

            # ---------------- A matrices ----------------
            # ta = tg - sum_k Rg[:,:, :,k] * J_rest[:,:,k]
            ta = sm.tile([BL, 72], F32)
            ta3 = ta[:].rearrange("p (j k) -> p j k", k=3)
            acc = sm.tile([BL, 72], F32)
            acc3 = acc[:].rearrange("p (j k) -> p j k", k=3)
            for k in range(3):
                a_in = g4[:, :, :, k]  # [b, J, 3(m)]
                b_in = jr3[:, :, k].unsqueeze(2).broadcast_to([BL, J, 3])
                if k == 0:
                    nc.vector.tensor_mul(acc3, a_in, b_in)
                else:
                    nc.vector.tensor_mul(ta3, a_in, b_in)
                    nc.vector.tensor_add(acc3, acc3, ta3)
            nc.vector.tensor_sub(ta3, t3, acc3)

            a_full = sm.tile([BL, 384], F32)
            a4 = a_full[:].rearrange("p (j m n) -> p j m n", m=4, n=4)
            nc.vector.memset(a_full[:], 0.0)
            nc.vector.memset(a4[:, :, 3, 3], 1.0)
            nc.vector.tensor_copy(a4[:, :, 0:3, 0:3], g4)
            nc.vector.tensor_copy(a4[:, :, 0:3, 3], ta3)
            nc.sync.dma_start(a_d[:], a_full[:])

            # ---------------- pose_feature^T ----------------
            nc.vector.tensor_scalar_add(
                rot[:, 9:216].rearrange("p (j a) -> p j a", a=9)[:, :, 0:9:4],
                rot[:, 9:216].rearrange("p (j a) -> p j a", a=9)[:, :, 0:9:4],
                -1.0,
            )
            ps_p1 = ps0.tile([128, BL], F32)
            nc.tensor.transpose(ps_p1[:], rot[:, 9:137], id_t[:])
            nc.vector.tensor_copy(poseT1[:], ps_p1[:])
            ps_p2 = ps0.tile([PF - 128, BL], F32)
            nc.tensor.transpose(ps_p2[:], rot[:, 137:216], id_t[:])
            nc.vector.tensor_copy(poseT2[:], ps_p2[:])

        # ---------------- A^T slices for LBS ----------------
        with (
            tc.tile_pool(name="psT", bufs=3, space="PSUM") as psT,
            tc.tile_pool(name="trT", bufs=3) as trT,
        ):
            for m in range(3):
                for n in range(4):
                    ps_t = psT.tile([J, BL], F32, tag="ps_t")
                    nc.tensor.transpose(ps_t[:], a4[:, :, m, n], id_t[:])
                    tr_t = trT.tile([J, BL], F32R, tag="tr_t")
                    nc.scalar.copy(tr_t[:], ps_t[:])
                    nc.sync.dma_start(
                        amnT[32 * n : 32 * n + J, m * BL : (m + 1) * BL], tr_t[:]
                    )

        # ---------------- stages A (blendshapes) + B (LBS), interleaved ----------------
        vp3 = vp_buf[:].rearrange("p (v k) -> p v k", k=3)
        with (
            tc.tile_pool(name="streamA", bufs=3) as stA,
            tc.tile_pool(name="outA", bufs=2) as outA,
            tc.tile_pool(name="psA", bufs=2, space="PSUM") as psA,
            tc.tile_pool(name="outB", bufs=3) as outB,
            tc.tile_pool(name="tmpB", bufs=2) as tmpB,
            tc.tile_pool(name="psB", bufs=1, space="PSUM") as psB,
        ):
            a_state = {}

            def a_load(ci):
                sd = stA.tile([12, CH_A], F16, tag="sd")
                nc.sync.dma_start(sd[:], sdir_d[ci])
                pd1 = stA.tile([128, CH_A], F16, tag="pd1")
                nc.sync.dma_start(pd1[:], pdir_d[ci, 0:128])
                pd2 = stA.tile([PF - 128, CH_A], F16, tag="pd2")
                nc.sync.dma_start(pd2[:], pdir_d[ci, 128:PF])
                ps = psA.tile([BL, CH_A], F32, tag="ps")
                a_state[ci] = (sd, pd1, pd2, ps)

            def a_bet(ci):
                sd, pd1, pd2, ps = a_state[ci]
                for h0 in (0, 512):
                    hs = slice(h0, h0 + 512)
                    nc.tensor.matmul(
                        ps[:, hs], btT_r[:], sd[:, hs],
                        start=True, stop=False, skip_group_check=True,
                    )

            def a_vs(ci):
                _, _, _, ps = a_state[ci]
                vs_sb = outA.tile([BL, CH_A], F32, tag="vs_sb")
                nc.scalar.copy(vs_sb[:], ps[:])
                nc.sync.dma_start(vshaped_d[ci], vs_sb[:])

            def a_pose(ci, h0):
                sd, pd1, pd2, ps = a_state[ci]
                hs = slice(h0, h0 + 512)
                nc.tensor.matmul(
                    ps[:, hs], poseT1[:], pd1[:, hs],
                    start=False, stop=False, skip_group_check=True,
                )
                nc.tensor.matmul(
                    ps[:, hs], poseT2[:], pd2[:, hs],
                    start=False, stop=True, skip_group_check=True,
                )

            def a_fin(ci):
                _, _, _, ps = a_state.pop(ci)
                c0 = ci * CH_A
                nc.scalar.copy(vp_buf[:, c0 : c0 + CH_A], ps[:])

            b_state = {}

            def b_round(ci, m):
                c0 = ci * CH_B
                if m == 0:
                    verts_sb = outB.tile([BL, 3 * CH_B], F32, tag="verts")
                    b_state[ci] = verts_sb
                verts_sb = b_state[ci]
                vv = verts_sb[:].rearrange("p (v m) -> p v m", m=3)
                ps_T = psB.tile([BL, 4, CH_B], F32, tag="T")
                for n in range(4):
                    nc.tensor.matmul(
                        ps_T[:, n, :],
                        amnT[32 * n : 32 * n + J, m * BL : (m + 1) * BL],
                        wt_t[32 * n : 32 * n + J, c0 : c0 + CH_B],
                        start=True,
                        stop=True,
                        tile_position=(32 * n, 0),
                    )
                # consume PSUM early (all PSUM reads on DVE, right after the matmuls)
                p01 = tmpB.tile([BL, 2, CH_B], F32, tag="p01")
                vpA = vp_buf[:][
                    :, 3 * c0 : 3 * (c0 + CH_B)
                ].rearrange("p (v k) -> p v k", k=3)
                in1 = vpA[:, :, 0:2].transpose([0, 2, 1])
                nc.vector.tensor_mul(p01[:], ps_T[:, 0:2, :], in1)
                p2 = tmpB.tile([BL, CH_B], F32, tag="p2")
                nc.vector.tensor_mul(p2[:], ps_T[:, 2, :], vp3[:, c0 : c0 + CH_B, 2])
                s2 = tmpB.tile([BL, CH_B], F32, tag="s2")
                nc.vector.tensor_add(s2[:], ps_T[:, 3, :], p2[:])
                s1 = tmpB.tile([BL, CH_B], F32, tag="s1")
                nc.gpsimd.tensor_add(s1[:], p01[:, 0, :], p01[:, 1, :])
                nc.gpsimd.tensor_add(vv[:, :, m], s1[:], s2[:])
                if m == 2:
                    nc.sync.dma_start(verts_d[ci], verts_sb[:])
                    del b_state[ci]

            # ---- schedule: A-halves and B-m-rounds interleaved
            a_steps = []
            for ci in range(NA):
                a_steps.append(("load", ci))
                a_steps.append(("bet", ci))
                a_steps.append(("vs", ci))
                a_steps.append(("pose", ci, 0))
                a_steps.append(("pose", ci, 512))
                a_steps.append(("fin", ci))
            a_ptr = 0
            a_done = 0

            def emit_a():
                nonlocal_ns = {}
                return None

            def emit_next_a():
                global _unused
                return None

            def run_a_step():
                nonlocal a_ptr, a_done
                if a_ptr >= len(a_steps):
                    return False
                step = a_steps[a_ptr]
                a_ptr += 1
                if step[0] == "load":
                    a_load(step[1])
                elif step[0] == "bet":
                    a_bet(step[1])
                elif step[0] == "vs":
                    a_vs(step[1])
                elif step[0] == "pose":
                    a_pose(step[1], step[2])
                else:
                    a_fin(step[1])
                    a_done += 1
                return True

            for k in range(NB):
                need = min(NA, -(-3 * (k + 1) // 2) + 1)
                while a_done < need:
                    run_a_step()
                for m in range(3):
                    b_round(k, m)
                    run_a_step()
            while run_a_step():
                pass

    nc.finalize()
    return nc


_NC_CACHE = {}


def _get_nc(debug=False):
    key = bool(debug)
    if key not in _NC_CACHE:
        _NC_CACHE[key] = build_nc(debug=debug)
    return _NC_CACHE[key]


def _host_prep(betas, full_pose, v_template, shapedirs, posedirs, J_regressor, lbs_weights):
    betas = np.asarray(betas, np.float32)
    full_pose = np.asarray(full_pose, np.float32)
    v_template = np.asarray(v_template, np.float32)
    shapedirs = np.asarray(shapedirs, np.float32)
    posedirs = np.asarray(posedirs, np.float32)
    J_regressor = np.asarray(J_regressor, np.float32)
    lbs_weights = np.asarray(lbs_weights, np.float32)

    sd_flat = shapedirs.reshape(VK, 10)
    vt_flat = v_template.reshape(VK)
    vt_hi = vt_flat.astype(np.float16).astype(np.float32)
    vt_lo = vt_flat - vt_hi
    sdirT_aug = np.zeros((12, VKP), np.float16)
    sdirT_aug[0:10, 0:VK] = sd_flat.T.astype(np.float16)
    sdirT_aug[10, 0:VK] = vt_hi.astype(np.float16)
    sdirT_aug[11, 0:VK] = vt_lo.astype(np.float16)
    sdirT_aug = np.ascontiguousarray(
        sdirT_aug.reshape(12, NA, CH_A).transpose(1, 0, 2)
    )  # [NA, 12, CH_A] fp16
    jrs = np.einsum(
        "jv,vkl->ljk", J_regressor.astype(np.float64), shapedirs.astype(np.float64)
    ).reshape(10, 72)
    jt = (J_regressor.astype(np.float64) @ v_template[0].astype(np.float64)).reshape(1, 72)
    js2aug = np.ascontiguousarray(
        np.concatenate([jrs, jt, np.zeros((1, 72))], axis=0), np.float32
    )  # [12, 72]
    wT = np.zeros((128, VP), np.float32)
    for n in range(4):
        wT[32 * n : 32 * n + J, 0:V] = lbs_weights.T
    betas_aug = np.concatenate([betas, np.ones((B, 2), np.float32)], axis=1)  # [B, 12]
    ident = np.eye(128, dtype=np.float32)
    return betas_aug, full_pose, sdirT_aug, js2aug, wT, ident


def kernel(betas, full_pose, v_template, shapedirs, posedirs, J_regressor, lbs_weights, parents):
    betas_aug, full_pose, sdirT_aug, js2aug, wT, ident = _host_prep(
        betas, full_pose, v_template, shapedirs, posedirs, J_regressor, lbs_weights
    )
    pd = np.asarray(posedirs, np.float32)
    pdt = np.zeros((PF, VKP), np.float16)
    pdt[:, 0:VK] = pd.astype(np.float16)
    posedirs = np.ascontiguousarray(pdt.reshape(PF, NA, CH_A).transpose(1, 0, 2))  # [NA, PF, CH_A] fp16

    nc = _get_nc(debug=False)
    in_maps = []
    for i in range(NCORES):
        sl = slice(i * BL, (i + 1) * BL)
        in_maps.append(
            {
                "pose": np.ascontiguousarray(full_pose[sl]),
                "betas_aug": np.ascontiguousarray(betas_aug[sl]),
                "js2aug": js2aug,
                "sdirT_aug": sdirT_aug,
                "posedirs": posedirs,
                "wT": wT,
                "ident": ident,
            }
        )
    res = run_bass_kernel_spmd(nc, in_maps, list(range(NCORES)))

    verts = np.empty((B, V, 3), np.float32)
    v_shaped = np.empty((B, V, 3), np.float32)
    j_posed = np.empty((B, J, 3), np.float32)
    j_rest = np.empty((B, J, 3), np.float32)
    a_mats = np.empty((B, J, 4, 4), np.float32)
    for i, r in enumerate(res.results):
        sl = slice(i * BL, (i + 1) * BL)
        verts[sl] = (
            r["verts"].transpose(1, 0, 2).reshape(BL, VP, 3)[:, 0:V, :]
        )
        v_shaped[sl] = (
            r["v_shaped"].transpose(1, 0, 2).reshape(BL, VKP)[:, 0:VK].reshape(BL, V, 3)
        )
        j_posed[sl] = r["j_posed"].reshape(BL, J, 3)
        j_rest[sl] = r["j_rest"].reshape(BL, J, 3)
        a_mats[sl] = r["a_mats"].reshape(BL, J, 4, 4)
    return verts, j_posed, j_rest, a_mats, v_shaped


# revision 16
# speedup vs baseline: 1.7734x; 1.0255x over previous
"""SMPL body-model (shape/pose blendshapes + LBS) on 8 TRN2 NeuronCores.

Data-parallel over batch: 1024/8 = 128 rows per core, batch on SBUF
partitions everywhere. Heavy matmuls run in fp32r; the kinematic chain and
the per-vertex transform apply run on the vector engine in fp32.
"""

import sys

sys.path.insert(0, "/opt/trn_rl_repo")

import math
from contextlib import ExitStack

import numpy as np

import concourse.bass as bass
import concourse.tile as tile
from concourse import bacc, mybir
from concourse.bass_utils import run_bass_kernel_spmd

F32 = mybir.dt.float32
F32R = mybir.dt.float32r
F16 = mybir.dt.float16
ALU = mybir.AluOpType
AFT = mybir.ActivationFunctionType

B, V, J = 1024, 6890, 24
NCORES, BL = 8, 128
PF = (J - 1) * 9  # 207
VK = V * 3  # 20670
CH_A = 1024  # stage A chunk (columns of v*3, padded)
CH_B = 512  # stage B chunk (vertices, padded)
VP = 7168  # padded vertex count (14 * 512)
VKP = VP * 3  # 21504 = 21 * 1024
NA = VKP // CH_A  # 21 stage A chunks
NB = VP // CH_B  # 14 stage B chunks
SMPL_PARENTS = [-1, 0, 0, 0, 1, 2, 3, 4, 5, 6, 7, 8, 9, 9, 9, 12, 13, 14, 16, 17, 18, 19, 20, 21]

# (j0, nj, p0, pstep) groups with affine parent indexing, grouped by tree depth
CHAIN_GROUPS = [
    (1, 3, 0, 0),
    (4, 3, 1, 1),
    (7, 3, 4, 1),
    (10, 3, 7, 1),
    (13, 2, 9, 0),
    (15, 3, 12, 1),
    (18, 2, 16, 1),
    (20, 2, 18, 1),
    (22, 2, 20, 1),
]
# rel_joints only needs J_rest, so consecutive-parent runs can span depths
REL_GROUPS = [(1, 3, 0, 0), (4, 9, 1, 1), (13, 2, 9, 0), (15, 3, 12, 1), (18, 6, 16, 1)]


def _jsl(view, j0, nj, step):
    """view[:, j0 : j0+nj] with parent step 1 or broadcast (step 0)."""
    if step == 1:
        return view[:, j0 : j0 + nj]
    sl = view[:, j0 : j0 + 1]
    return sl.broadcast_to((sl.shape[0], nj) + tuple(sl.shape[2:]))


def build_nc(debug=False):
    nc = bacc.Bacc("TRN2", target_bir_lowering=False, debug=False, num_devices=NCORES)

    pose_d = nc.dram_tensor("pose", [BL, 72], F32, kind="ExternalInput")
    ba_d = nc.dram_tensor("betas_aug", [BL, 12], F32, kind="ExternalInput")
    js2_d = nc.dram_tensor("js2aug", [12, 72], F32, kind="ExternalInput")
    sdir_d = nc.dram_tensor("sdirT_aug", [NA, 12, CH_A], F16, kind="ExternalInput")
    pdir_d = nc.dram_tensor("posedirs", [NA, PF, CH_A], F16, kind="ExternalInput")
    wt_d = nc.dram_tensor("wT", [128, VP], F32, kind="ExternalInput")
    id_d = nc.dram_tensor("ident", [128, 128], F32, kind="ExternalInput")

    verts_d = nc.dram_tensor("verts", [NB, BL, 3 * CH_B], F32, kind="ExternalOutput")
    vshaped_d = nc.dram_tensor("v_shaped", [NA, BL, CH_A], F32, kind="ExternalOutput")
    jposed_d = nc.dram_tensor("j_posed", [BL, 72], F32, kind="ExternalOutput")
    jrest_d = nc.dram_tensor("j_rest", [BL, 72], F32, kind="ExternalOutput")
    a_d = nc.dram_tensor("a_mats", [BL, 384], F32, kind="ExternalOutput")
    if debug:
        rot_dbg = nc.dram_tensor("rot_dbg", [BL, 216], F32, kind="ExternalOutput")
        rg_dbg = nc.dram_tensor("rg_dbg", [BL, 216], F32, kind="ExternalOutput")

    with tile.TileContext(nc) as tc, ExitStack() as ctx:
        keep = ctx.enter_context(tc.tile_pool(name="keep", bufs=1))
        sm = ctx.enter_context(tc.tile_pool(name="small", bufs=1))

        # ---------------- loads ----------------
        pose_t = sm.tile([BL, 72], F32)
        nc.sync.dma_start(pose_t[:], pose_d[:])
        ba_t = sm.tile([BL, 12], F32)
        nc.sync.dma_start(ba_t[:], ba_d[:])
        id_t = keep.tile([128, 128], F32)
        nc.sync.dma_start(id_t[:], id_d[:])
        js2_t = sm.tile([12, 72], F32)
        nc.sync.dma_start(js2_t[:], js2_d[:])
        wt_t = keep.tile([128, VP], F32R)
        nc.sync.dma_start(wt_t[:], wt_d[:].bitcast(F32R))

        vp_buf = keep.tile([BL, VKP], F32)  # v_posed, resident (padded)
        btT_r = keep.tile([12, BL], F16)
        poseT1 = keep.tile([128, BL], F16)
        poseT2 = keep.tile([PF - 128, BL], F16)
        amnT = keep.tile([128, 3 * BL], F32R)  # per m: 4 row-packed [24,128] lhsT

        with tc.tile_pool(name="ps0", bufs=1, space="PSUM") as ps0:
            # betas^T (for J_rest fp32 matmul and stage A fp32r lhsT)
            ps_bt = ps0.tile([12, BL], F32)
            nc.tensor.transpose(ps_bt[:], ba_t[:], id_t[:])
            btT_f = sm.tile([12, BL], F32)
            nc.scalar.copy(btT_f[:], ps_bt[:])
            nc.vector.tensor_copy(btT_r[:], ps_bt[:])

            # J_rest = betas_aug @ js2aug  (exact fp32)
            ps_jr = ps0.tile([BL, 72], F32)
            nc.tensor.matmul(ps_jr[:], btT_f[:], js2_t[:], start=True, stop=True)
            jrest_t = sm.tile([BL, 72], F32)
            nc.scalar.copy(jrest_t[:], ps_jr[:])
            nc.sync.dma_start(jrest_d[:], jrest_t[:])

            # ---------------- Rodrigues ----------------
            rv8 = sm.tile([BL, 72], F32)
            nc.vector.tensor_scalar_add(rv8[:], pose_t[:], 1e-8)
            sq = sm.tile([BL, 72], F32)
            nc.vector.tensor_mul(sq[:], rv8[:], rv8[:])
            n2 = sm.tile([BL, J], F32)
            nc.vector.reduce_sum(
                n2[:], sq[:].rearrange("p (j k) -> p j k", k=3), axis=mybir.AxisListType.X
            )
            ang = sm.tile([BL, J], F32)
            nc.scalar.activation(ang[:], n2[:], AFT.Sqrt)
            inv = sm.tile([BL, J], F32)
            nc.vector.reciprocal(inv[:], ang[:])
            s_t = sm.tile([BL, J], F32)
            nc.scalar.activation(s_t[:], ang[:], AFT.Sin)
            c_t = sm.tile([BL, J], F32)
            halfpi = sm.tile([BL, 1], F32)
            nc.vector.memset(halfpi[:], math.pi / 2)
            nc.scalar.activation(c_t[:], ang[:], AFT.Sin, bias=halfpi[:])

            axis_t = sm.tile([BL, 72], F32)  # [b, (j,3)]
            ax3 = axis_t[:].rearrange("p (j k) -> p j k", k=3)
            nc.vector.tensor_mul(
                ax3,
                pose_t[:].rearrange("p (j k) -> p j k", k=3),
                inv[:].unsqueeze(2).broadcast_to([BL, J, 3]),
            )

            rot = sm.tile([BL, 216], F32)  # [b, (j,m,n)] local rotations
            r4 = rot[:].rearrange("p (j m n) -> p j m n", m=3, n=3)
            # (1-c) * outer(axis, axis)
            omc = sm.tile([BL, J], F32)
            nc.vector.tensor_scalar(omc[:], c_t[:], -1.0, 1.0, op0=ALU.mult, op1=ALU.add)
            nc.vector.tensor_mul(
                r4,
                ax3.unsqueeze(3).broadcast_to([BL, J, 3, 3]),
                ax3.unsqueeze(2).broadcast_to([BL, J, 3, 3]),
            )
            r9 = rot[:].rearrange("p (j a) -> p j a", a=9)
            nc.vector.tensor_mul(
                r9, r9, omc[:].unsqueeze(2).broadcast_to([BL, J, 9])
            )
            # + c on the diagonal
            nc.vector.tensor_add(
                r9[:, :, 0:9:4], r9[:, :, 0:9:4], c_t[:].unsqueeze(2).broadcast_to([BL, J, 3])
            )
            # +/- s*axis off-diagonals
            sa = sm.tile([BL, 72], F32)
            sa3 = sa[:].rearrange("p (j k) -> p j k", k=3)
            nc.vector.tensor_mul(sa3, ax3, s_t[:].unsqueeze(2).broadcast_to([BL, J, 3]))
            for pos, comp, sign in ((1, 2, -1), (3, 2, 1), (2, 1, 1), (6, 1, -1), (5, 0, -1), (7, 0, 1)):
                op = nc.vector.tensor_add if sign > 0 else nc.vector.tensor_sub
                op(r9[:, :, pos], r9[:, :, pos], sa3[:, :, comp])
            if debug:
                rot_sb = sm.tile([BL, 216], F32)
                nc.vector.tensor_copy(rot_sb[:], rot[:])
                nc.sync.dma_start(rot_dbg[:], rot_sb[:])

            # ---------------- rel joints ----------------
            rel = sm.tile([BL, 72], F32)
            jr3 = jrest_t[:].rearrange("p (j k) -> p j k", k=3)
            rl3 = rel[:].rearrange("p (j k) -> p j k", k=3)
            nc.vector.tensor_copy(rl3[:, 0:1], jr3[:, 0:1])
            for j0, nj, p0, pstep in REL_GROUPS:
                nc.vector.tensor_sub(
                    rl3[:, j0 : j0 + nj], jr3[:, j0 : j0 + nj], _jsl(jr3, p0, nj, pstep)
                )

            # ---------------- kinematic chain ----------------
            rg = sm.tile([BL, 216], F32)
            tg = sm.tile([BL, 72], F32)
            g4 = rg[:].rearrange("p (j m n) -> p j m n", m=3, n=3)
            t3 = tg[:].rearrange("p (j k) -> p j k", k=3)
            nc.vector.tensor_copy(rg[:, 0:9], rot[:, 0:9])
            nc.vector.tensor_copy(t3[:, 0:1], rl3[:, 0:1])
            tmpR = sm.tile([BL, 27], F32)
            tmpt = sm.tile([BL, 9], F32)
            for j0, nj, p0, pstep in CHAIN_GROUPS:
                dstR = g4[:, j0 : j0 + nj]  # [b, nj, m, n]
                locR = r4[:, j0 : j0 + nj]
                parR = _jsl(g4, p0, nj, pstep)
                tR = tmpR[:].rearrange("p (j m n) -> p j m n", m=3, n=3)[:, 0:nj]
                for k in range(3):
                    a_in = parR[:, :, :, k].unsqueeze(3).broadcast_to([BL, nj, 3, 3])
                    b_in = locR[:, :, k, :].unsqueeze(2).broadcast_to([BL, nj, 3, 3])
                    if k == 0:
                        nc.vector.tensor_mul(dstR, a_in, b_in)
                    else:
                        nc.vector.tensor_mul(tR, a_in, b_in)
                        nc.vector.tensor_add(dstR, dstR, tR)
                # translations
                dstT = t3[:, j0 : j0 + nj]
                locT = rl3[:, j0 : j0 + nj]
                tT = tmpt[:].rearrange("p (j k) -> p j k", k=3)[:, 0:nj]
                for k in range(3):
                    a_in = parR[:, :, :, k]  # [b, nj, 3(m)]
                    b_in = locT[:, :, k].unsqueeze(2).broadcast_to([BL, nj, 3])
                    if k == 0:
                        nc.vector.tensor_mul(dstT, a_in, b_in)
                    else:
                        nc.vector.tensor_mul(tT, a_in, b_in)
                        nc.vector.tensor_add(dstT, dstT, tT)
                nc.vector.tensor_add(dstT, dstT, _jsl(t3, p0, nj, pstep))

            nc.sync.dma_start(jposed_d[:], tg[:])
            if debug:
                rg_sb = sm.tile([BL, 216], F32)
                nc.vector.tensor_copy(rg_sb[:], rg[:])
                nc.sync.dma_start(rg_dbg[:], rg_sb[:])

            # ---------------- A matrices ----------------
            # ta = tg - sum_k Rg[:,:, :,k] * J_rest[:,:,k]
            ta = sm.tile([BL, 72], F32)
            ta3 = ta[:].rearrange("p (j k) -> p j k", k=3)
            acc = sm.tile([BL, 72], F32)
            acc3 = acc[:].rearrange("p (j k) -> p j k", k=3)
            for k in range(3):
                a_in = g4[:, :, :, k]  # [b, J, 3(m)]
                b_in = jr3[:, :, k].unsqueeze(2).broadcast_to([BL, J, 3])
                if k == 0:
                    nc.vector.tensor_mul(acc3, a_in, b_in)
                else:
                    nc.vector.tensor_mul(ta3, a_in, b_in)
                    nc.vector.tensor_add(acc3, acc3, ta3)
            nc.vector.tensor_sub(ta3, t3, acc3)

            a_full = sm.tile([BL, 384], F32)
            a4 = a_full[:].rearrange("p (j m n) -> p j m n", m=4, n=4)
            nc.vector.memset(a_full[:], 0.0)
            nc.vector.memset(a4[:, :, 3, 3], 1.0)
            nc.vector.tensor_copy(a4[:, :, 0:3, 0:3], g4)
            nc.vector.tensor_copy(a4[:, :, 0:3, 3], ta3)
            nc.sync.dma_start(a_d[:], a_full[:])

            # ---------------- pose_feature^T ----------------
            nc.vector.tensor_scalar_add(
                rot[:, 9:216].rearrange("p (j a) -> p j a", a=9)[:, :, 0:9:4],
                rot[:, 9:216].rearrange("p (j a) -> p j a", a=9)[:, :, 0:9:4],
                -1.0,
            )
            ps_p1 = ps0.tile([128, BL], F32)
            nc.tensor.transpose(ps_p1[:], rot[:, 9:137], id_t[:])
            nc.vector.tensor_copy(poseT1[:], ps_p1[:])
            ps_p2 = ps0.tile([PF - 128, BL], F32)
            nc.tensor.transpose(ps_p2[:], rot[:, 137:216], id_t[:])
            nc.vector.tensor_copy(poseT2[:], ps_p2[:])

        # ---------------- A^T slices for LBS ----------------
        with (
            tc.tile_pool(name="psT", bufs=3, space="PSUM") as psT,
            tc.tile_pool(name="trT", bufs=3) as trT,
        ):
            for m in range(3):
                for n in range(4):
                    ps_t = psT.tile([J, BL], F32, tag="ps_t")
                    nc.tensor.transpose(ps_t[:], a4[:, :, m, n], id_t[:])
                    tr_t = trT.tile([J, BL], F32R, tag="tr_t")
                    nc.scalar.copy(tr_t[:], ps_t[:])
                    nc.sync.dma_start(
                        amnT[32 * n : 32 * n + J, m * BL : (m + 1) * BL], tr_t[:]
                    )

        # ---------------- stages A (blendshapes) + B (LBS), interleaved ----------------
        vp3 = vp_buf[:].rearrange("p (v k) -> p v k", k=3)
        with (
            tc.tile_pool(name="streamA", bufs=3) as stA,
            tc.tile_pool(name="outA", bufs=2) as outA,
            tc.tile_pool(name="psA", bufs=1, space="PSUM") as psA,
            tc.tile_pool(name="outB", bufs=2) as outB,
            tc.tile_pool(name="tmpB", bufs=2) as tmpB,
            tc.tile_pool(name="psB", bufs=1, space="PSUM") as psB,
            tc.tile_pool(name="psJ", bufs=2, space="PSUM") as psJ,
        ):
            junk_rhs = keep.tile([128, 256], F16)
            nc.vector.memset(junk_rhs[:], 0.5)

            def pe_filler(n=1):
                for _ in range(n):
                    jp = psJ.tile([128, 256], F32, tag="junk")
                    nc.tensor.matmul(
                        jp[:], poseT1[:, 0:128], junk_rhs[:], start=True, stop=True
                    )
            a_state = {}

            def a_load(ci):
                sd = stA.tile([12, CH_A], F16, tag="sd")
                nc.sync.dma_start(sd[:], sdir_d[ci])
                pd1 = stA.tile([128, CH_A], F16, tag="pd1")
                nc.sync.dma_start(pd1[:], pdir_d[ci, 0:128])
                pd2 = stA.tile([PF - 128, CH_A], F16, tag="pd2")
                nc.sync.dma_start(pd2[:], pdir_d[ci, 128:PF])
                ps = psA.tile([BL, CH_A], F32, tag="ps")
                a_state[ci] = (sd, pd1, pd2, ps)

            def a_bet(ci):
                sd, pd1, pd2, ps = a_state[ci]
                for h0 in (0, 512):
                    hs = slice(h0, h0 + 512)
                    nc.tensor.matmul(
                        ps[:, hs], btT_r[:], sd[:, hs],
                        start=True, stop=False, skip_group_check=True,
                    )

            def a_vs(ci):
                _, _, _, ps = a_state[ci]
                vs_sb = outA.tile([BL, CH_A], F32, tag="vs_sb")
                nc.scalar.copy(vs_sb[:], ps[:])
                nc.sync.dma_start(vshaped_d[ci], vs_sb[:])

            def a_pose(ci, h0):
                sd, pd1, pd2, ps = a_state[ci]
                hs = slice(h0, h0 + 512)
                nc.tensor.matmul(
                    ps[:, hs], poseT1[:], pd1[:, hs],
                    start=False, stop=False, skip_group_check=True,
                )
                nc.tensor.matmul(
                    ps[:, hs], poseT2[:], pd2[:, hs],
                    start=False, stop=True, skip_group_check=True,
                )

            def a_fin(ci):
                _, _, _, ps = a_state.pop(ci)
                c0 = ci * CH_A
                nc.scalar.copy(vp_buf[:, c0 : c0 + CH_A], ps[:])

            b_state = {}

            def b_round(ci, m):
                c0 = ci * CH_B
                if m == 0:
                    verts_sb = outB.tile([BL, 3 * CH_B], F32, tag="verts")
                    b_state[ci] = verts_sb
                verts_sb = b_state[ci]
                vv = verts_sb[:].rearrange("p (v m) -> p v m", m=3)
                ps_T = psB.tile([BL, 4, CH_B], F32, tag="T")
                for n in range(4):
                    nc.tensor.matmul(
                        ps_T[:, n, :],
                        amnT[32 * n : 32 * n + J, m * BL : (m + 1) * BL],
                        wt_t[32 * n : 32 * n + J, c0 : c0 + CH_B],
                        start=True,
                        stop=True,
                        tile_position=(32 * n, 0),
                    )
                # consume PSUM fast: one fused DVE mult + ACT copy of the translation
                p012 = tmpB.tile([BL, 3, CH_B], F32, tag="p012")
                vpA = vp_buf[:][
                    :, 3 * c0 : 3 * (c0 + CH_B)
                ].rearrange("p (v k) -> p v k", k=3)
                in1 = vpA[:, :, 0:3].transpose([0, 2, 1])
                nc.vector.tensor_mul(p012[:], ps_T[:, 0:3, :], in1)
                t3c = tmpB.tile([BL, CH_B], F32, tag="t3c")
                nc.scalar.copy(t3c[:], ps_T[:, 3, :])
                s2 = tmpB.tile([BL, CH_B], F32, tag="s2")
                nc.vector.tensor_add(s2[:], p012[:, 2, :], t3c[:])
                s1 = tmpB.tile([BL, CH_B], F32, tag="s1")
                nc.gpsimd.tensor_add(s1[:], p012[:, 0, :], p012[:, 1, :])
                nc.gpsimd.tensor_add(vv[:, :, m], s1[:], s2[:])
                if m == 2:
                    nc.sync.dma_start(verts_d[ci], verts_sb[:])
                    del b_state[ci]

            # ---- schedule: A-halves and B-m-rounds interleaved
            a_steps = []
            for ci in range(NA):
                a_steps.append(("load", ci))
                a_steps.append(("bet", ci))
                a_steps.append(("vs", ci))
                a_steps.append(("pose", ci, 0))
                a_steps.append(("pose", ci, 512))
                a_steps.append(("fin", ci))
            a_ptr = 0
            a_done = 0

            def emit_a():
                nonlocal_ns = {}
                return None

            def emit_next_a():
                global _unused
                return None

            def run_a_step():
                nonlocal a_ptr, a_done
                if a_ptr >= len(a_steps):
                    return False
                step = a_steps[a_ptr]
                a_ptr += 1
                if step[0] == "load":
                    a_load(step[1])
                elif step[0] == "bet":
                    a_bet(step[1])
                elif step[0] == "vs":
                    a_vs(step[1])
                elif step[0] == "pose":
                    a_pose(step[1], step[2])
                else:
                    a_fin(step[1])
                    a_done += 1
                return True

            for k in range(NB):
                need = min(NA, -(-3 * (k + 1) // 2) + 1)
                while a_done < need:
                    run_a_step()
                    pe_filler(1)
                for m in range(3):
                    b_round(k, m)
                    run_a_step()
                    pe_filler(2)
            while run_a_step():
                pe_filler(2)

    nc.finalize()
    return nc


_NC_CACHE = {}


def _get_nc(debug=False):
    key = bool(debug)
    if key not in _NC_CACHE:
        _NC_CACHE[key] = build_nc(debug=debug)
    return _NC_CACHE[key]


def _host_prep(betas, full_pose, v_template, shapedirs, posedirs, J_regressor, lbs_weights):
    betas = np.asarray(betas, np.float32)
    full_pose = np.asarray(full_pose, np.float32)
    v_template = np.asarray(v_template, np.float32)
    shapedirs = np.asarray(shapedirs, np.float32)
    posedirs = np.asarray(posedirs, np.float32)
    J_regressor = np.asarray(J_regressor, np.float32)
    lbs_weights = np.asarray(lbs_weights, np.float32)

    sd_flat = shapedirs.reshape(VK, 10)
    vt_flat = v_template.reshape(VK)
    vt_hi = vt_flat.astype(np.float16).astype(np.float32)
    vt_lo = vt_flat - vt_hi
    sdirT_aug = np.zeros((12, VKP), np.float16)
    sdirT_aug[0:10, 0:VK] = sd_flat.T.astype(np.float16)
    sdirT_aug[10, 0:VK] = vt_hi.astype(np.float16)
    sdirT_aug[11, 0:VK] = vt_lo.astype(np.float16)
    sdirT_aug = np.ascontiguousarray(
        sdirT_aug.reshape(12, NA, CH_A).transpose(1, 0, 2)
    )  # [NA, 12, CH_A] fp16
    jrs = np.einsum(
        "jv,vkl->ljk", J_regressor.astype(np.float64), shapedirs.astype(np.float64)
    ).reshape(10, 72)
    jt = (J_regressor.astype(np.float64) @ v_template[0].astype(np.float64)).reshape(1, 72)
    js2aug = np.ascontiguousarray(
        np.concatenate([jrs, jt, np.zeros((1, 72))], axis=0), np.float32
    )  # [12, 72]
    wT = np.zeros((128, VP), np.float32)
    for n in range(4):
        wT[32 * n : 32 * n + J, 0:V] = lbs_weights.T
    betas_aug = np.concatenate([betas, np.ones((B, 2), np.float32)], axis=1)  # [B, 12]
    ident = np.eye(128, dtype=np.float32)
    return betas_aug, full_pose, sdirT_aug, js2aug, wT, ident


def kernel(betas, full_pose, v_template, shapedirs, posedirs, J_regressor, lbs_weights, parents):
    betas_aug, full_pose, sdirT_aug, js2aug, wT, ident = _host_prep(
        betas, full_pose, v_template, shapedirs, posedirs, J_regressor, lbs_weights
    )
    pd = np.asarray(posedirs, np.float32)
    pdt = np.zeros((PF, VKP), np.float16)
    pdt[:, 0:VK] = pd.astype(np.float16)
    posedirs = np.ascontiguousarray(pdt.reshape(PF, NA, CH_A).transpose(1, 0, 2))  # [NA, PF, CH_A] fp16

    nc = _get_nc(debug=False)
    in_maps = []
    for i in range(NCORES):
        sl = slice(i * BL, (i + 1) * BL)
        in_maps.append(
            {
                "pose": np.ascontiguousarray(full_pose[sl]),
                "betas_aug": np.ascontiguousarray(betas_aug[sl]),
                "js2aug": js2aug,
                "sdirT_aug": sdirT_aug,
                "posedirs": posedirs,
                "wT": wT,
                "ident": ident,
            }
        )
    res = run_bass_kernel_spmd(nc, in_maps, list(range(NCORES)))

    verts = np.empty((B, V, 3), np.float32)
    v_shaped = np.empty((B, V, 3), np.float32)
    j_posed = np.empty((B, J, 3), np.float32)
    j_rest = np.empty((B, J, 3), np.float32)
    a_mats = np.empty((B, J, 4, 4), np.float32)
    for i, r in enumerate(res.results):
        sl = slice(i * BL, (i + 1) * BL)
        verts[sl] = (
            r["verts"].transpose(1, 0, 2).reshape(BL, VP, 3)[:, 0:V, :]
        )
        v_shaped[sl] = (
            r["v_shaped"].transpose(1, 0, 2).reshape(BL, VKP)[:, 0:VK].reshape(BL, V, 3)
        )
        j_posed[sl] = r["j_posed"].reshape(BL, J, 3)
        j_rest[sl] = r["j_rest"].reshape(BL, J, 3)
        a_mats[sl] = r["a_mats"].reshape(BL, J, 4, 4)
    return verts, j_posed, j_rest, a_mats, v_shaped
